# Initial kernel scaffold
#
"""Trainium2 Bass kernel for nn_EnhancedTransformerBlock (8-core Megatron TP).

v2 design notes (cost-model-driven rewrite of the working baseline):
- Weights are int8-quantized ON HOST and shipped as bf16 (exact for |v|<=127):
  kills the on-device weight absmax/quantize passes and halves weight HBM
  traffic. Weight scales ride in a tiny f32 consts tensor.
- norm1 is REPLICATED (every core normalizes+quantizes the full residual
  stream locally): no AllGather and no stat AllReduces for layer 1.
- All cross-core max-reductions use small AllGather + local reduce (15us)
  instead of AllReduce (28us).
- Attention computed in transposed orientation: scoresT[k,q] = kk^T qq so the
  softmax denominator is a matmul ones-row and attn weights feed the AV
  matmul without PE transposes. Rel-pos bias is injected into the scores
  PSUM via a diag(1/(sq*sk)) identity matmul (zero elementwise cost).
- WO / MLP reduce-scatters run in bf16; norm2 stats (ssq+colmax) share one
  packed AllGather; x2q crosses cores as int8 in 4 seq-chunked AllGathers
  pipelined under the MLP-up matmul.
"""
import os
from contextlib import ExitStack

import ml_dtypes
import numpy as np

import concourse.bass as bass  # noqa: F401  (engine registry import side effects)
import concourse.mybir as mybir
import concourse.tile as tile
from concourse import bacc, bass_isa
from concourse.bass_utils import run_bass_kernel_spmd
from concourse.masks import make_identity

P = 128
S = 1024
DM = 4096
MLP = 16384
NCORE = 8
HL = 8            # heads per core
DHL = 512         # local qkv/feature shard (HL*64)
MLPL = 2048       # local mlp cols
FT = DM // P      # 32
FTL = DHL // P    # 4
MT = MLPL // P    # 16
TBL = 1920        # rel-pos diag table row length
SF = float(np.sqrt(64.0) * 1024.0 ** 0.25)
MAGIC = 12582912.0  # 1.5*2^23: (x+M)-M == rint(x) for |x|<2^22
NCHUNK = 4        # x2q allgather seq chunks (small first so MLP starts early)
CB = [(0, 128), (128, 384), (384, 640), (640, 1024)]

F32 = mybir.dt.float32
BF16 = mybir.dt.bfloat16
I8 = mybir.dt.int8
AL = mybir.AluOpType
AF = mybir.ActivationFunctionType
AX = mybir.AxisListType
RG = [list(range(NCORE))]
BF = ml_dtypes.bfloat16

_cache = {}
last_results = None


class _Stop(Exception):
    pass


def _bias_src(tbl_dr, h, kt0=0, nkt=4):
    """[P, nkt, S] view of head h's table: (p, kt, q) -> tbl[h*P+p, q + 896 - (kt0+kt)*128].

    With TT[h, p, m] = rel[clip(m - p - 864, 0, 64), h] this reads
    biasT[k0+p, q] = rel[clip(q - (k0+p) + 32)] for k-tile kt (k0 = kt*128).
    """
    src = tbl_dr[:, :].copy()
    v = src.ap
    v[0] = (TBL, P)
    v[1] = (-P, nkt)
    v.append((1, S))
    src.ap = v
    src.offset = h * P * TBL + 896 - kt0 * P
    return src


def _phase_limit():
    v = os.environ.get("KERNEL_PHASE")
    return int(v) if v else 99


def _build(dbg=False, phase=99):
    nc = bacc.Bacc("TRN2", target_bir_lowering=False, debug=False, num_devices=NCORE)
    io = {"_dbg": dbg, "_phase": phase}

    def di(name, shape, dt=F32):
        io[name] = nc.dram_tensor(name, shape, dt, kind="ExternalInput")

    di("xb", [DM, S], BF16)          # full input, transposed, bf16 (norm1 path)
    di("x32", [DHL, S])              # own feature shard fp32 (residual path)
    di("sc1", [DM]); di("sc2", [DHL])
    di("wqb", [DM, DHL], BF16); di("wkb", [DM, DHL], BF16); di("wvb", [DM, DHL], BF16)
    di("bq", [DHL]); di("bk", [DHL]); di("bv", [DHL])
    di("tbl", [HL * P, TBL], BF16)
    di("wob", [DHL, DM], BF16); di("bo", [DHL])
    di("w1b", [DM, MLPL], BF16); di("b1", [MLPL])
    di("w2b", [MLPL, DM], BF16); di("b2", [DHL])
    di("wsc", [1, 8])                # host weight scales: swq swk swv swo sw1 sw2
    io["out"] = nc.dram_tensor("out", [DHL, S], F32, kind="ExternalOutput")
    if dbg:
        io["dbg_xq"] = nc.dram_tensor("dbg_xq", [DM, S], BF16, kind="ExternalOutput")
        io["dbg_q"] = nc.dram_tensor("dbg_q", [DHL, S], BF16, kind="ExternalOutput")
        io["dbg_k"] = nc.dram_tensor("dbg_k", [DHL, S], BF16, kind="ExternalOutput")
        io["dbg_v"] = nc.dram_tensor("dbg_v", [S, DHL], BF16, kind="ExternalOutput")
        io["dbg_ao"] = nc.dram_tensor("dbg_ao", [DHL, S], BF16, kind="ExternalOutput")
        io["dbg_x2"] = nc.dram_tensor("dbg_x2", [DHL, S], F32, kind="ExternalOutput")
        io["dbg_h"] = nc.dram_tensor("dbg_h", [MLPL, S], BF16, kind="ExternalOutput")
        io["dbg_x2q"] = nc.dram_tensor("dbg_x2q", [DM, S], I8, kind="ExternalOutput")
        io["dbg_sc"] = nc.dram_tensor("dbg_sc", [1, 16], F32, kind="ExternalOutput")

    with tile.TileContext(nc) as tc:
        _body(nc, tc, io)
    nc.compile()
    return nc


def _body(nc, tc, io):
    dbg = io["_dbg"]
    _phase = io["_phase"]

    top = ExitStack()
    _stacks = [top]
    const = top.enter_context(tc.tile_pool(name="const", bufs=1))
    dram = top.enter_context(tc.tile_pool(name="dram", bufs=1, space="DRAM"))

    ones_bf = const.tile([P, 1], BF16)
    nc.vector.memset(ones_bf[:, :], 1.0)
    ident = const.tile([P, P], F32)
    make_identity(nc, ident)

    def load_vec(dr, n_tiles, name):
        t = const.tile([P, n_tiles], F32, name=name)
        nc.scalar.dma_start(t[:, :], dr[:].rearrange("(o p) -> p o", p=P))
        return t

    sc1_sb = load_vec(io["sc1"], FT, "sc1_sb")
    sc2_sb = load_vec(io["sc2"], FTL, "sc2_sb")
    bq_sb = load_vec(io["bq"], FTL, "bq_sb")
    bk_sb = load_vec(io["bk"], FTL, "bk_sb")
    bo_sb = load_vec(io["bo"], FTL, "bo_sb")
    b1_sb = load_vec(io["b1"], MT, "b1_sb")
    b2_sb = load_vec(io["b2"], FTL, "b2_sb")
    wsc_sb = const.tile([1, 8], F32, name="wsc_sb")
    nc.scalar.dma_start(wsc_sb[:, :], io["wsc"][:, :])
    bv_row = const.tile([1, DHL], F32, name="bv_row")
    nc.scalar.dma_start(bv_row[:, :], io["bv"][:].unsqueeze(0))
    bv_bc = const.tile([P, DHL], F32, name="bv_bc")
    nc.gpsimd.partition_broadcast(bv_bc[:, :], bv_row[:, :], channels=P)

    def sc11(name):
        return const.tile([1, 1], F32, name=name)

    def bc(src11, name, ch=P):
        t = const.tile([ch, 1], F32, name=name)
        nc.gpsimd.partition_broadcast(t[:, :], src11, channels=ch)
        return t

    def quant_scale(mx11, name):
        """s = mx/127 + 1e-8; returns (s, 1/s) [1,1] tiles."""
        s = sc11(name + "_s")
        nc.vector.tensor_scalar(s[:, :], mx11, 1.0 / 127.0, 1e-8, AL.mult, AL.add)
        inv = sc11(name + "_inv")
        nc.vector.reciprocal(inv[:, :], s[:, :])
        return s, inv

    def smul(a11, b11, name):
        t = sc11(name)
        nc.vector.tensor_tensor(t[:, :], a11, b11, AL.mult)
        return t

    def agmax(vals, tag, q=None):
        """AllGather-based global max of [1,1] APs -> list of [1,1] tiles."""
        n = len(vals)
        loc = const.tile([1, n], F32, name=f"agl_{tag}")
        for i, v in enumerate(vals):
            nc.vector.tensor_copy(loc[:, i:i + 1], v)
        ag_in = dram.tile([1, n], F32, name=f"agi_{tag}")
        ag_out = dram.tile([NCORE, n], F32, addr_space="Shared", name=f"ago_{tag}")
        (q or nc.scalar).dma_start(ag_in[:, :], loc[:, :])
        nc.gpsimd.collective_compute("AllGather", AL.bypass, replica_groups=RG,
                                     ins=[ag_in[:, :].opt()], outs=[ag_out[:, :].opt()])
        g = const.tile([NCORE, n], F32, name=f"agg_{tag}")
        nc.gpsimd.dma_start(g[:, :], ag_out[:, :])
        r = const.tile([NCORE, n], F32, name=f"agr_{tag}")
        nc.gpsimd.partition_all_reduce(r[:, :], g[:, :], channels=NCORE,
                                       reduce_op=bass_isa.ReduceOp.max)
        return [r[:1, i:i + 1] for i in range(n)]

    def _ckpt(n):
        if _phase <= n:
            raise _Stop()

    try:
        # long-lived pools, opened in reverse order of their close points
        ao_cm = ExitStack(); _stacks.append(ao_cm)
        aop = ao_cm.enter_context(tc.tile_pool(name="aop", bufs=1))
        ao_sb = aop.tile([64, HL, S], BF16, name="ao_sb")
        wow_cm = ExitStack(); _stacks.append(wow_cm)
        wow = wow_cm.enter_context(tc.tile_pool(name="wow", bufs=1))
        wo_sb = wow.tile([P, FTL, DM], BF16, name="wo_sb")
        attn_cm = ExitStack(); _stacks.append(attn_cm)
        attnk = attn_cm.enter_context(tc.tile_pool(name="attnk", bufs=1))
        qq_sb = attnk.tile([P, FTL, S], BF16, name="qq_sb")
        kk_sb = attnk.tile([P, FTL, S], BF16, name="kk_sb")
        vq_ext = attnk.tile([P, 8, 8, 66], BF16, name="vq_ext")
        nc.vector.memset(vq_ext[:, :, :, :], 1.0)
        xq_cm = ExitStack(); _stacks.append(xq_cm)
        xqp = xq_cm.enter_context(tc.tile_pool(name="xqp", bufs=1))
        xq_all = xqp.tile([P, FT, S], BF16, name="xq_all")

        # ================= Phase A: norm1, replicated =================
        with tc.tile_pool(name="n1k", bufs=1) as n1k, \
             tc.tile_pool(name="n1t", bufs=2) as n1t, \
             tc.tile_pool(name="n1st", bufs=1) as n1st, \
             tc.tile_pool(name="n1ps", bufs=1, space="PSUM") as n1ps:
            cmax = n1k.tile([P, S], BF16, name="cmax")
            u1_bc = n1k.tile([P, S], BF16, name="u1_bc")
            ssq_ps = n1ps.tile([1, 2, 512], F32, name="ssq_ps")
            for g in range(FT // 4):
                xt = n1t.tile([P, 4, S], BF16, tag="xt", name="xt")
                nc.sync.dma_start(xt[:, :, :],
                                  io["xb"][g * 4 * P:(g + 1) * 4 * P, :]
                                  .rearrange("(o p) f -> p o f", p=P))
                for j in range(4):
                    t = g * 4 + j
                    sq = n1t.tile([P, S], BF16, tag="sq", name="sq")
                    nc.scalar.activation(sq[:, :], xt[:, j, :], AF.Square)
                    for n in range(2):
                        nc.tensor.matmul(ssq_ps[:, n, :], ones_bf[:, :],
                                         sq[:, n * 512:(n + 1) * 512],
                                         start=(t == 0), stop=(t == FT - 1))
                    nc.vector.tensor_scalar(xq_all[:, t, :], xt[:, j, :],
                                            sc1_sb[:, t:t + 1], None, AL.mult)
                    ab = n1t.tile([P, S], BF16, tag="ab", name="ab")
                    nc.scalar.activation(ab[:, :], xq_all[:, t, :], AF.Abs)
                    if t == 0:
                        nc.vector.tensor_copy(cmax[:, :], ab[:, :])
                    else:
                        nc.vector.tensor_tensor(cmax[:, :], cmax[:, :], ab[:, :],
                                                AL.max)

            cmf = n1st.tile([P, S], F32, name="cmf")
            nc.vector.tensor_copy(cmf[:, :], cmax[:, :])
            cmr = n1st.tile([P, S], F32, name="cmr")
            nc.gpsimd.partition_all_reduce(cmr[:, :], cmf[:, :], channels=P,
                                           reduce_op=bass_isa.ReduceOp.max)
            ssq = n1st.tile([1, S], F32, name="ssq")
            nc.scalar.copy(ssq[:, :], ssq_ps[:, :, :].rearrange("p a b -> p (a b)"))
            rstd = n1st.tile([1, S], F32, name="rstd1")
            nc.vector.tensor_scalar(rstd[:, :], ssq[:, :], 1.0 / DM, 1e-6,
                                    AL.mult, AL.add)
            nc.scalar.activation(rstd[:, :], rstd[:, :], AF.Sqrt)
            nc.vector.reciprocal(rstd[:, :], rstd[:, :])
            sxv = n1st.tile([1, S], F32, name="sxv")
            nc.vector.tensor_tensor(sxv[:, :], cmr[:1, :], rstd[:, :], AL.mult)
            mx1 = sc11("mx1")
            nc.vector.tensor_reduce(mx1[:, :], sxv[:, :], AX.X, AL.max)
            sx1, inv_sx1 = quant_scale(mx1[:, :], "sx1")
            u1 = n1st.tile([1, S], BF16, name="u1")
            nc.vector.tensor_scalar(u1[:, :], rstd[:, :], inv_sx1[:, :], None, AL.mult)
            nc.gpsimd.partition_broadcast(u1_bc[:, :], u1[:, :], channels=P)
            # in-place quantize: xq_all currently holds xs = x*sc1
            for t in range(FT):
                nc.vector.tensor_tensor(xq_all[:, t, :], xq_all[:, t, :],
                                        u1_bc[:, :], AL.mult)
                nc.vector.tensor_scalar(xq_all[:, t, :], xq_all[:, t, :],
                                        MAGIC, MAGIC, AL.add, AL.subtract)
        if dbg:
            nc.sync.dma_start(io["dbg_xq"][:, :].rearrange("(o p) f -> p o f", p=P),
                              xq_all[:, :, :])
        _ckpt(1)

        # ================= Phase B: QKV projections =================
        nc.sync.dma_start(wo_sb[:, :, :],
                          io["wob"][:, :].rearrange("(g p) c -> p g c", p=P))
        aq_bc = bc(smul(sx1[:, :], wsc_sb[:, 0:1], "aq")[:, :], "aq_bc")
        ak_bc = bc(smul(sx1[:, :], wsc_sb[:, 1:2], "ak")[:, :], "ak_bc")
        av_bc = bc(smul(sx1[:, :], wsc_sb[:, 2:3], "av")[:, :], "av_bc")

        vv_cm = ExitStack(); _stacks.append(vv_cm)
        vvf = vv_cm.enter_context(tc.tile_pool(name="vvf", bufs=1))
        v_bf = vvf.tile([P, 8, DHL], BF16, name="v_bf")
        qk_cm = ExitStack(); _stacks.append(qk_cm)
        qkf = qk_cm.enter_context(tc.tile_pool(name="qkf", bufs=1))
        q_bf = qkf.tile([P, FTL, S], BF16, name="q_bf")
        k_bf = qkf.tile([P, FTL, S], BF16, name="k_bf")

        qmaxs = const.tile([P, 3], F32, name="qkv_max")
        with tc.tile_pool(name="wld", bufs=2) as wldp, \
             tc.tile_pool(name="qkev", bufs=3) as qev, \
             tc.tile_pool(name="qkvps", bufs=1, space="PSUM") as qkv_ps:
            for wi, (which, w_dr, alpha, bias_sb, dest) in enumerate((
                    ("q", io["wqb"], aq_bc, bq_sb, q_bf),
                    ("k", io["wkb"], ak_bc, bk_sb, k_bf))):
                pss = [qkv_ps.tile([P, 512], F32, tag=f"ps{i}", name=f"ps_{which}{i}")
                       for i in range(8)]
                for k0 in range(0, FT, 4):
                    wb = wldp.tile([P, 4, DHL], BF16, tag="wqk", name=f"w_{which}")
                    nc.sync.dma_start(wb[:, :, :],
                                      w_dr[k0 * P:(k0 + 4) * P, :]
                                      .rearrange("(g p) c -> p g c", p=P))
                    for g in range(4):
                        k = k0 + g
                        for m in range(FTL):
                            for n in range(2):
                                nc.tensor.matmul(pss[m * 2 + n][:, :],
                                                 wb[:, g, m * P:(m + 1) * P],
                                                 xq_all[:, k, n * 512:(n + 1) * 512],
                                                 start=(k == 0), stop=(k == FT - 1))
                red = qev.tile([P, FTL, 2], F32, tag=f"red{which}", name=f"red_{which}")
                for m in range(FTL):
                    for n in range(2):
                        nc.scalar.activation(dest[:, m, n * 512:(n + 1) * 512],
                                             pss[m * 2 + n][:, :], AF.Identity,
                                             bias=bias_sb[:, m:m + 1],
                                             scale=alpha[:, :1])
                        nc.vector.tensor_reduce(red[:, m, n:n + 1],
                                                dest[:, m, n * 512:(n + 1) * 512],
                                                AX.X, AL.max, apply_absolute_value=True)
                nc.vector.tensor_reduce(qmaxs[:, wi:wi + 1], red[:, :, :], AX.XY, AL.max)

            # launch q/k max allgather while V matmuls run
            parq = const.tile([P, 2], F32, name="parqk")
            nc.gpsimd.partition_all_reduce(parq[:, :], qmaxs[:, 0:2], channels=P,
                                           reduce_op=bass_isa.ReduceOp.max)
            gq, gk = agmax([parq[:1, 0:1], parq[:1, 1:2]], "qk")
            sq_s, invq = quant_scale(gq, "sq")
            sk_s, invk = quant_scale(gk, "sk")
            invq_bc, invk_bc = bc(invq[:, :], "invq_bc"), bc(invk[:, :], "invk_bc")
            sqk = smul(sq_s[:, :], sk_s[:, :], "sqk")
            alpha = sc11("alpha")
            nc.vector.tensor_scalar(alpha[:, :], sqk[:, :], 1.0 / SF, None, AL.mult)
            alpha_bc = bc(alpha[:, :], "alpha_bc")
            inv_sqk = sc11("inv_sqk")
            nc.vector.reciprocal(inv_sqk[:, :], sqk[:, :])
            inv_sqk_bc = bc(inv_sqk[:, :], "inv_sqk_bc")
            identc = const.tile([P, P], BF16, name="identc")
            nc.vector.tensor_scalar(identc[:, :], ident[:, :], inv_sqk_bc[:, :1],
                                    None, AL.mult)

            pss_v = [qkv_ps.tile([P, 512], F32, tag=f"ps{i}", name=f"ps_v{i}")
                     for i in range(8)]
            for k0 in range(0, FT, 4):
                wb = wldp.tile([P, 4, DHL], BF16, tag="wqk", name="w_v")
                nc.sync.dma_start(wb[:, :, :],
                                  io["wvb"][k0 * P:(k0 + 4) * P, :]
                                  .rearrange("(g p) c -> p g c", p=P))
                for g in range(4):
                    k = k0 + g
                    for m in range(8):
                        nc.tensor.matmul(pss_v[m][:, :],
                                         xq_all[:, k, m * P:(m + 1) * P],
                                         wb[:, g, :],
                                         start=(k == 0), stop=(k == FT - 1))
            vred = qev.tile([P, 8], F32, tag="vred", name="vred")
            for m in range(8):
                ev = qev.tile([P, DHL], F32, tag="vev", name="vev")
                nc.scalar.mul(ev[:, :], pss_v[m][:, :], av_bc[:, :1])
                nc.vector.tensor_tensor(v_bf[:, m, :], ev[:, :], bv_bc[:, :], AL.add)
                nc.vector.tensor_reduce(vred[:, m:m + 1], v_bf[:, m, :], AX.X,
                                        AL.max, apply_absolute_value=True)
            nc.vector.tensor_reduce(qmaxs[:, 2:3], vred[:, :], AX.X, AL.max)
            parv = const.tile([P, 1], F32, name="parv")
            nc.gpsimd.partition_all_reduce(parv[:, :], qmaxs[:, 2:3], channels=P,
                                           reduce_op=bass_isa.ReduceOp.max)
            (gv,) = agmax([parv[:1, :]], "v")
        if dbg:
            nc.sync.dma_start(io["dbg_q"][:, :].rearrange("(o p) f -> p o f", p=P),
                              q_bf[:, :, :])
            nc.sync.dma_start(io["dbg_k"][:, :].rearrange("(o p) f -> p o f", p=P),
                              k_bf[:, :, :])
            nc.sync.dma_start(io["dbg_v"][:, :].rearrange("(o p) f -> p o f", p=P),
                              v_bf[:, :, :])
        _ckpt(2)

        # ================= Phase C: quantize q/k/v =================
        with tc.tile_pool(name="qknt", bufs=2) as qknt:
            for t in range(FTL):
                for src, dst, ibc in ((q_bf, qq_sb, invq_bc), (k_bf, kk_sb, invk_bc)):
                    tmp = qknt.tile([P, S], BF16, tag="qkq", name="qkq")
                    nc.scalar.mul(tmp[:, :], src[:, t, :], ibc[:, :1])
                    nc.vector.tensor_scalar(dst[:, t, :], tmp[:, :], MAGIC, MAGIC,
                                            AL.add, AL.subtract)
        qk_cm.close()
        _stacks.remove(qk_cm)
        with tc.tile_pool(name="vqnt", bufs=1) as vqnt:
            sv_s, invv = quant_scale(gv, "sv")
            invv_bc = bc(invv[:, :], "invv_bc")
            tmp = vqnt.tile([P, 8, DHL], BF16, tag="vq", name="vqt")
            nc.vector.tensor_scalar(tmp[:, :, :], v_bf[:, :, :], invv_bc[:, :1],
                                    None, AL.mult)
            nc.vector.tensor_scalar(
                vq_ext[:, :, :, 0:64],
                tmp[:, :, :].rearrange("p m (h d) -> p m h d", h=8),
                MAGIC, MAGIC, AL.add, AL.subtract)
        vv_cm.close()
        _stacks.remove(vv_cm)
        xq_cm.close()
        _stacks.remove(xq_cm)
        _ckpt(3)

        # ================= Phase D: attention =================
        aomax = const.tile([64, HL], F32, name="aomax")
        with tc.tile_pool(name="att", bufs=2) as att, \
             tc.tile_pool(name="attb", bufs=2) as attb, \
             tc.tile_pool(name="scps", bufs=2, space="PSUM") as scps, \
             tc.tile_pool(name="avps", bufs=2, space="PSUM") as avps:
            for h in range(HL):
                pb = 64 * (h % 2)
                ht = h // 2
                bias_t = att.tile([P, 8, S], BF16, tag="bias", name="bias")
                nc.scalar.dma_start(bias_t[:, 0:4, :], _bias_src(io["tbl"], h, 0))
                nc.scalar.dma_start(bias_t[:, 4:8, :], _bias_src(io["tbl"], h, 4))
                attnT = att.tile([P, 8, S], BF16, tag="attnT", name="attnT")
                avp = avps.tile([P, 2, 512], F32, tag="avp", name="avp")
                for kt in range(8):
                    ps = scps.tile([P, 2, 512], F32, tag="sc", name="sc_ps")
                    for n in range(2):
                        nc.tensor.matmul(ps[:, n, :],
                                         kk_sb[pb:pb + 64, ht, kt * P:(kt + 1) * P],
                                         qq_sb[pb:pb + 64, ht, n * 512:(n + 1) * 512],
                                         start=True, stop=False)
                        nc.tensor.matmul(ps[:, n, :], identc[:, :],
                                         bias_t[:, kt, n * 512:(n + 1) * 512],
                                         start=False, stop=True)
                    nc.scalar.activation(attnT[:, kt, :],
                                         ps[:, :, :].rearrange("p a b -> p (a b)"),
                                         AF.Exp, scale=alpha_bc[:, :1])
                    for n in range(2):
                        nc.tensor.matmul(avp[:65, n, :],
                                         vq_ext[:, kt, h, 0:65],
                                         attnT[:, kt, n * 512:(n + 1) * 512],
                                         start=(kt == 0), stop=(kt == 7))
                den = attb.tile([1, S], F32, tag="den", name="den")
                nc.vector.tensor_scalar(den[:, :],
                                        avp[64:65, :, :].rearrange("p a b -> p (a b)"),
                                        1e-6, None, AL.add)
                nc.vector.reciprocal(den[:, :], den[:, :])
                rbc = attb.tile([64, S], F32, tag="rbc", name="rbc")
                nc.gpsimd.partition_broadcast(rbc[:, :], den[:, :], channels=64)
                for n in range(2):
                    nc.vector.tensor_tensor(ao_sb[:, h, n * 512:(n + 1) * 512],
                                            avp[:64, n, :],
                                            rbc[:, n * 512:(n + 1) * 512], AL.mult)
                nc.vector.tensor_reduce(aomax[:, h:h + 1], ao_sb[:, h, :], AX.X,
                                        AL.max, apply_absolute_value=True)
        attn_cm.close()
        _stacks.remove(attn_cm)
        if dbg:
            nc.sync.dma_start(
                io["dbg_ao"][:, :].rearrange("(h d) f -> d h f", h=HL), ao_sb[:, :, :])

        aored = const.tile([64, 1], F32, name="aored")
        nc.vector.tensor_reduce(aored[:, :], aomax[:, :], AX.X, AL.max)
        aopar = const.tile([64, 1], F32, name="aopar")
        nc.gpsimd.partition_all_reduce(aopar[:, :], aored[:, :], channels=64,
                                       reduce_op=bass_isa.ReduceOp.max)
        (graw,) = agmax([aopar[:1, :]], "ao", q=nc.sync)
        _ckpt(4)

        # ================= Phase E: quantize ao, WO matmul, RS =================
        s_ao = sc11("s_ao")
        nc.vector.tensor_tensor(s_ao[:, :], sv_s[:, :], graw, AL.mult)
        nc.vector.tensor_scalar(s_ao[:, :], s_ao[:, :], 1.0 / 127.0, 1e-8,
                                AL.mult, AL.add)
        inv_sao = sc11("inv_sao")
        nc.vector.reciprocal(inv_sao[:, :], s_ao[:, :])
        m_ao_bc = bc(smul(sv_s[:, :], inv_sao[:, :], "m_ao")[:, :], "m_ao_bc", ch=64)
        a_wo_bc = bc(smul(s_ao[:, :], wsc_sb[:, 3:4], "a_wo")[:, :], "a_wo_bc")

        wo_cm = ExitStack(); _stacks.append(wo_cm)
        wop = wo_cm.enter_context(tc.tile_pool(name="wop", bufs=1))
        aoq_sb = wop.tile([P, FTL, S], BF16, name="aoq_sb")
        x2_cm = ExitStack(); _stacks.append(x2_cm)
        x2p = x2_cm.enter_context(tc.tile_pool(name="x2p", bufs=1))
        x2_sb = x2p.tile([P, FTL, S], F32, name="x2_sb")
        x2q_cm = ExitStack(); _stacks.append(x2q_cm)
        x2qp = x2q_cm.enter_context(tc.tile_pool(name="x2qp", bufs=1))
        x2q_i8 = x2qp.tile([P, FTL, S], I8, name="x2q_i8")
        x32_cm = ExitStack(); _stacks.append(x32_cm)
        x32p = x32_cm.enter_context(tc.tile_pool(name="x32p", bufs=1))
        x32_sb = x32p.tile([P, FTL, S], F32, name="x32_sb")
        nc.sync.dma_start(x32_sb[:, :, :],
                          io["x32"][:, :].rearrange("(o p) f -> p o f", p=P))

        with tc.tile_pool(name="aoqt", bufs=2) as aoqt:
            for h in range(HL):
                tmp = aoqt.tile([64, S], BF16, tag="aoq", name="aoqh")
                nc.scalar.mul(tmp[:, :], ao_sb[:, h, :], m_ao_bc[:, :1])
                if h % 2 == 0:
                    nc.vector.tensor_scalar(aoq_sb[0:64, h // 2, :], tmp[:, :],
                                            MAGIC, MAGIC, AL.add, AL.subtract)
                else:
                    tmp2 = aoqt.tile([64, S], BF16, tag="aoq2", name="aoqh2")
                    nc.vector.tensor_scalar(tmp2[:, :], tmp[:, :],
                                            MAGIC, MAGIC, AL.add, AL.subtract)
                    nc.gpsimd.dma_start(aoq_sb[64:128, h // 2, :], tmp2[:, :])

        aout_d = [dram.tile([DM, 512], BF16, name=f"aout{n}") for n in range(2)]
        rs_d = [dram.tile([DHL, 512], BF16, name=f"aors{n}") for n in range(2)]
        with tc.tile_pool(name="woev", bufs=3) as woev, \
             tc.tile_pool(name="wops", bufs=2, space="PSUM") as wops:
            for n in range(2):
                for mg in range(8):
                    ps = wops.tile([P, 4, 512], F32, tag="wops", name="wo_ps")
                    for k in range(FTL):
                        for mi in range(4):
                            m = mg * 4 + mi
                            nc.tensor.matmul(ps[:, mi, :],
                                             wo_sb[:, k, m * P:(m + 1) * P],
                                             aoq_sb[:, k, n * 512:(n + 1) * 512],
                                             start=(k == 0), stop=(k == FTL - 1))
                    ev = woev.tile([P, 4, 512], BF16, tag="woev", name="wo_ev")
                    nc.scalar.mul(ev[:, :, :], ps[:, :, :], a_wo_bc[:, :1])
                    nc.sync.dma_start(
                        aout_d[n][mg * 4 * P:(mg + 1) * 4 * P, :]
                        .rearrange("(g p) c -> p g c", p=P), ev[:, :, :])
                nc.gpsimd.collective_compute("ReduceScatter", AL.add, replica_groups=RG,
                                             ins=[aout_d[n][:, :].opt()],
                                             outs=[rs_d[n][:, :].opt()])

        # x2 build fused with norm2 stats (per seq-half, right behind each RS)
        with tc.tile_pool(name="x2t", bufs=2) as x2t, \
             tc.tile_pool(name="n2t", bufs=2) as n2t, \
             tc.tile_pool(name="n2ps", bufs=1, space="PSUM") as n2ps:
            ssq2_ps = n2ps.tile([1, 2, 512], F32, name="ssq2_ps")
            cm2 = n2t.tile([P, S], F32, tag="cm2", name="cm2")
            for n in range(2):
                h0, h1 = n * 512, (n + 1) * 512
                rst = x2t.tile([P, FTL, 512], BF16, tag="rst", name="rst")
                nc.sync.dma_start(rst[:, :, :],
                                  rs_d[n][:, :].rearrange("(o p) f -> p o f", p=P))
                for t in range(FTL):
                    tmp = x2t.tile([P, 512], F32, tag="x2tmp", name="x2tmp")
                    nc.vector.tensor_scalar(tmp[:, :], rst[:, t, :],
                                            bo_sb[:, t:t + 1], None, AL.add)
                    nc.vector.tensor_tensor(x2_sb[:, t, h0:h1], tmp[:, :],
                                            x32_sb[:, t, h0:h1], AL.add)
                    sq = n2t.tile([P, 512], BF16, tag="sq2", name="sq2")
                    nc.scalar.activation(sq[:, :], x2_sb[:, t, h0:h1], AF.Square)
                    nc.tensor.matmul(ssq2_ps[:, n, :], ones_bf[:, :], sq[:, :],
                                     start=(t == 0), stop=(t == FTL - 1))
                    xs2 = n2t.tile([P, 512], F32, tag="xs2", name="xs2")
                    nc.vector.tensor_scalar(xs2[:, :], x2_sb[:, t, h0:h1],
                                            sc2_sb[:, t:t + 1], None, AL.mult)
                    ab2 = n2t.tile([P, 512], F32, tag="ab2", name="ab2")
                    nc.scalar.activation(ab2[:, :], xs2[:, :], AF.Abs)
                    if t == 0:
                        nc.vector.tensor_copy(cm2[:, h0:h1], ab2[:, :])
                    else:
                        nc.vector.tensor_tensor(cm2[:, h0:h1], cm2[:, h0:h1],
                                                ab2[:, :], AL.max)
            cm2r_x = x2qp.tile([P, S], F32, name="cm2r_x")
            nc.gpsimd.partition_all_reduce(cm2r_x[:, :], cm2[:, :], channels=P,
                                           reduce_op=bass_isa.ReduceOp.max)
            ssq2_row = x2qp.tile([1, S], F32, name="ssq2_row")
            nc.scalar.copy(ssq2_row[:, :],
                           ssq2_ps[:, :, :].rearrange("p a b -> p (a b)"))
        x32_cm.close()
        _stacks.remove(x32_cm)
        if dbg:
            nc.sync.dma_start(io["dbg_x2"][:, :].rearrange("(o p) f -> p o f", p=P),
                              x2_sb[:, :, :])
        _ckpt(5)

        # ================= Phase F: norm2 reduce + quantize + chunked AG ====
        agx_in = [dram.tile([DHL, c1 - c0], I8, name=f"agx_in{c}")
                  for c, (c0, c1) in enumerate(CB)]
        agx_out = [dram.tile([DM, c1 - c0], I8, addr_space="Shared", name=f"agx_out{c}")
                   for c, (c0, c1) in enumerate(CB)]
        with tc.tile_pool(name="n2b", bufs=2) as n2t:
            pack = n2t.tile([1, 2 * S], F32, tag="pack", name="pack")
            nc.vector.tensor_copy(pack[:, 0:S], ssq2_row[:, :])
            nc.vector.tensor_copy(pack[:, S:2 * S], cm2r_x[:1, :])
            st_in = dram.tile([1, 2 * S], F32, name="st_in")
            st_out = dram.tile([NCORE, 2 * S], F32, addr_space="Shared", name="st_out")
            nc.scalar.dma_start(st_in[:, :], pack[:, :])
            nc.gpsimd.collective_compute("AllGather", AL.bypass, replica_groups=RG,
                                         ins=[st_in[:, :].opt()],
                                         outs=[st_out[:, :].opt()])
            stg = n2t.tile([NCORE, 2 * S], F32, tag="stg", name="stg")
            nc.gpsimd.dma_start(stg[:, :], st_out[:, :])
            ssq2g = n2t.tile([NCORE, S], F32, tag="ssq2g", name="ssq2g")
            nc.gpsimd.partition_all_reduce(ssq2g[:, :], stg[:, 0:S], channels=NCORE,
                                           reduce_op=bass_isa.ReduceOp.add)
            cm2g = n2t.tile([NCORE, S], F32, tag="cm2g", name="cm2g")
            nc.gpsimd.partition_all_reduce(cm2g[:, :], stg[:, S:2 * S], channels=NCORE,
                                           reduce_op=bass_isa.ReduceOp.max)
            rstd2 = n2t.tile([1, S], F32, tag="rstd2", name="rstd2")
            nc.vector.tensor_scalar(rstd2[:, :], ssq2g[:1, :], 1.0 / DM, 1e-6,
                                    AL.mult, AL.add)
            nc.scalar.activation(rstd2[:, :], rstd2[:, :], AF.Sqrt)
            nc.vector.reciprocal(rstd2[:, :], rstd2[:, :])
            sxv2 = n2t.tile([1, S], F32, tag="sxv2", name="sxv2")
            nc.vector.tensor_tensor(sxv2[:, :], cm2g[:1, :], rstd2[:, :], AL.mult)
            mx2 = sc11("mx2")
            nc.vector.tensor_reduce(mx2[:, :], sxv2[:, :], AX.X, AL.max)
            sx2, inv_sx2 = quant_scale(mx2[:, :], "sx2")
            u2 = n2t.tile([1, S], F32, tag="u2", name="u2")
            nc.vector.tensor_scalar(u2[:, :], rstd2[:, :], inv_sx2[:, :], None, AL.mult)
            u2_bc = n2t.tile([P, S], F32, tag="u2bc", name="u2_bc")
            nc.gpsimd.partition_broadcast(u2_bc[:, :], u2[:, :], channels=P)
            # quantize + allgather chunk by chunk so the first AG fires early
            for c, (c0, c1) in enumerate(CB):
                for t in range(FTL):
                    xs2 = n2t.tile([P, c1 - c0], F32, tag="xs2b", name="xs2b")
                    nc.vector.tensor_scalar(xs2[:, :], x2_sb[:, t, c0:c1],
                                            sc2_sb[:, t:t + 1], None, AL.mult)
                    xnq = n2t.tile([P, c1 - c0], F32, tag="xnq", name="xnq")
                    nc.vector.tensor_tensor(xnq[:, :], xs2[:, :],
                                            u2_bc[:, c0:c1], AL.mult)
                    xqb = n2t.tile([P, c1 - c0], BF16, tag="xqb", name="xqb")
                    nc.vector.tensor_scalar(xqb[:, :], xnq[:, :], MAGIC, MAGIC,
                                            AL.add, AL.subtract)
                    nc.vector.tensor_copy(x2q_i8[:, t, c0:c1], xqb[:, :])
                nc.scalar.dma_start(
                    agx_in[c][:, :].rearrange("(o p) f -> p o f", p=P),
                    x2q_i8[:, :, c0:c1])
                nc.gpsimd.collective_compute("AllGather", AL.bypass, replica_groups=RG,
                                             ins=[agx_in[c][:, :].opt()],
                                             outs=[agx_out[c][:, :].opt()])
        _ckpt(6)
        x2q_cm.close()
        _stacks.remove(x2q_cm)
        # spill x2 (final residual input) to free SBUF for the MLP weights.
        # On the ACT queue: on SP it head-of-line blocks the w1 prefetch.
        x2_dram = dram.tile([DHL, S], F32, name="x2_dram")
        nc.scalar.dma_start(x2_dram[:, :].rearrange("(o p) f -> p o f", p=P),
                            x2_sb[:, :, :])
        x2_cm.close()
        _stacks.remove(x2_cm)
        wo_cm.close()
        _stacks.remove(wo_cm)
        wow_cm.close()
        _stacks.remove(wow_cm)
        ao_cm.close()
        _stacks.remove(ao_cm)

        # ================= Phase G: MLP up =================
        a1_bc = bc(smul(sx2[:, :], wsc_sb[:, 4:5], "a1")[:, :], "a1_bc")
        hmax = const.tile([P, MT], F32, name="hmax")

        h_cm = ExitStack(); _stacks.append(h_cm)
        hp = h_cm.enter_context(tc.tile_pool(name="hp", bufs=1))
        h_sb = hp.tile([P, MT, S], BF16, name="h_sb")
        w1_cm = ExitStack(); _stacks.append(w1_cm)
        w1p = w1_cm.enter_context(tc.tile_pool(name="w1p", bufs=1))
        w1_sb = w1p.tile([P, FT, MLPL], BF16, name="w1_sb")
        # groups 0-1 land on addresses WAR-pinned by x2 until its last read;
        # load them last and rotate the k-loop so they are consumed last too
        for g in (3, 4, 5, 6, 7):
            k0 = g * 4
            nc.sync.dma_start(w1_sb[:, k0:k0 + 4, :],
                              io["w1b"][k0 * P:(k0 + 4) * P, :]
                              .rearrange("(g p) c -> p g c", p=P))
        # groups 0-2 sit on addresses pinned until ~norm2-quant; small pieces
        # so the x2q staging transfer is not stuck behind a long backlog
        for g in (0, 1, 2):
            for half in range(2):
                k0 = g * 4 + half * 2
                nc.sync.dma_start(w1_sb[:, k0:k0 + 2, :],
                                  io["w1b"][k0 * P:(k0 + 2) * P, :]
                                  .rearrange("(g p) c -> p g c", p=P))

        with tc.tile_pool(name="m1s", bufs=1) as m1s, \
             tc.tile_pool(name="m1ps", bufs=2, space="PSUM") as m1ps:
            for c, (c0, c1) in enumerate(CB):
                cw = c1 - c0
                xgb = m1s.tile([P, FT, 256 if c % 2 == 0 else 384], BF16,
                               tag="xgbA" if c % 2 == 0 else "xgbB", name="xgb")
                nc.gpsimd.dma_start(
                    xgb[:, :, 0:cw],
                    agx_out[c][:, :].rearrange("(o p) f -> p o f", p=P))
                for mg in range(4):
                    # [P, 4, 512] so each mi region owns a full 2KB PSUM bank:
                    # matmul start=True clears at bank granularity.
                    ps = m1ps.tile([P, 4, 512], F32, tag="m1ps", name="m1_ps")
                    for ki in range(FT):
                        k = (ki + 12) % FT
                        for mi in range(4):
                            m = mg * 4 + mi
                            nc.tensor.matmul(ps[:, mi, 0:cw],
                                             w1_sb[:, k, m * P:(m + 1) * P],
                                             xgb[:, k, 0:cw],
                                             start=(ki == 0), stop=(ki == FT - 1))
                    for mi in range(4):
                        m = mg * 4 + mi
                        nc.scalar.activation(h_sb[:, m, c0:c1], ps[:, mi, 0:cw],
                                             AF.Gelu_apprx_tanh,
                                             bias=b1_sb[:, m:m + 1], scale=a1_bc[:, :1])
                    nc.vector.tensor_reduce(
                        hmax[:, c * 4 + mg:c * 4 + mg + 1],
                        h_sb[:, mg * 4:(mg + 1) * 4, c0:c1], AX.XY, AL.max,
                        apply_absolute_value=True)
        w1_cm.close()
        _stacks.remove(w1_cm)
        if dbg:
            nc.sync.dma_start(io["dbg_h"][:, :].rearrange("(o p) f -> p o f", p=P),
                              h_sb[:, :, :])

        hred = const.tile([P, 1], F32, name="hred")
        nc.vector.tensor_reduce(hred[:, :], hmax[:, :], AX.X, AL.max)
        hpar = const.tile([P, 1], F32, name="hpar")
        nc.gpsimd.partition_all_reduce(hpar[:, :], hred[:, :], channels=P,
                                       reduce_op=bass_isa.ReduceOp.max)
        (gh,) = agmax([hpar[:1, :]], "h", q=nc.sync)
        _ckpt(7)

        # ================= Phase H: MLP down =================
        sh_s, invh = quant_scale(gh, "sh")
        invh_bc = bc(invh[:, :], "invh_bc")
        a2_bc = bc(smul(sh_s[:, :], wsc_sb[:, 5:6], "a2")[:, :], "a2_bc")

        w2_cm = ExitStack(); _stacks.append(w2_cm)
        w2p = w2_cm.enter_context(tc.tile_pool(name="w2p", bufs=1))
        w2_sb = w2p.tile([P, MT, DM], BF16, name="w2_sb")
        for mg in range(8):
            nc.sync.dma_start(
                w2_sb[:, :, mg * 512:(mg + 1) * 512],
                io["w2b"][:, mg * 512:(mg + 1) * 512]
                .rearrange("(g p) c -> p g c", p=P))
        # quantize h in place (values become the int8 grid in bf16);
        # spread across DVE and gpsimd so the serial chain is shorter
        for g in range(MT // 4):
            sl = h_sb[:, g * 4:(g + 1) * 4, :]
            eng = nc.gpsimd if g == 3 else nc.vector
            eng.tensor_scalar(sl, sl, invh_bc[:, :1], None, AL.mult)
            eng.tensor_scalar(sl, sl, MAGIC, MAGIC, AL.add, AL.subtract)
        hq_sb = h_sb

        y_d = [dram.tile([DM, 256], BF16, name=f"y{n}") for n in range(4)]
        yrs_d = [dram.tile([DHL, 256], BF16, name=f"yrs{n}") for n in range(4)]
        with tc.tile_pool(name="m2ev", bufs=3) as m2ev, \
             tc.tile_pool(name="m2ps", bufs=2, space="PSUM") as m2ps:
            for n in range(4):
                for mg in range(8):
                    ps = m2ps.tile([P, 4, 512], F32, tag="m2ps", name="m2_ps")
                    for k in range(MT):
                        for mi in range(4):
                            m = mg * 4 + mi
                            nc.tensor.matmul(ps[:, mi, 0:256],
                                             w2_sb[:, k, m * P:(m + 1) * P],
                                             hq_sb[:, k, n * 256:(n + 1) * 256],
                                             start=(k == 0), stop=(k == MT - 1))
                    ev = m2ev.tile([P, 4, 256], BF16, tag="m2ev", name="m2_ev")
                    nc.scalar.mul(ev[:, :, :], ps[:, :, 0:256], a2_bc[:, :1])
                    nc.sync.dma_start(
                        y_d[n][mg * 4 * P:(mg + 1) * 4 * P, :]
                        .rearrange("(g p) c -> p g c", p=P), ev[:, :, :])
                nc.gpsimd.collective_compute("ReduceScatter", AL.add, replica_groups=RG,
                                             ins=[y_d[n][:, :].opt()],
                                             outs=[yrs_d[n][:, :].opt()])
        w2_cm.close()
        _stacks.remove(w2_cm)
        h_cm.close()
        _stacks.remove(h_cm)
        _ckpt(8)

        if dbg:
            scs = [sx1[:, :], sq_s[:, :], sk_s[:, :], sv_s[:, :], s_ao[:, :],
                   sx2[:, :], sh_s[:, :], alpha[:, :], inv_sqk[:, :], graw, gh,
                   gq, gk, gv, mx1[:, :], mx2[:, :]]
            scv = const.tile([1, 16], F32, name="dbg_scv")
            for i, s in enumerate(scs):
                nc.vector.tensor_copy(scv[:, i:i + 1], s)
            nc.sync.dma_start(io["dbg_sc"][:, :], scv[:, :])

        with tc.tile_pool(name="fint", bufs=2) as fint:
            for n in range(4):
                n0, n1 = n * 256, (n + 1) * 256
                yt = fint.tile([P, FTL, 256], BF16, tag="yrst", name="yrst")
                nc.sync.dma_start(yt[:, :, :],
                                  yrs_d[n][:, :].rearrange("(o p) f -> p o f", p=P))
                x2r = fint.tile([P, FTL, 256], F32, tag="x2r", name="x2r")
                nc.sync.dma_start(x2r[:, :, :],
                                  x2_dram[:, n0:n1]
                                  .rearrange("(o p) f -> p o f", p=P))
                out_t = fint.tile([P, FTL, 256], F32, tag="outt", name="outt")
                for t in range(FTL):
                    tmp = fint.tile([P, 256], F32, tag="fin", name="fin")
                    nc.vector.tensor_scalar(tmp[:, :], yt[:, t, :],
                                            b2_sb[:, t:t + 1], None, AL.add)
                    nc.vector.tensor_tensor(out_t[:, t, :], tmp[:, :],
                                            x2r[:, t, :], AL.add)
                nc.sync.dma_start(
                    io["out"][:, n0:n1]
                    .rearrange("(o p) f -> p o f", p=P), out_t[:, :, :])

    except _Stop:
        pass
    finally:
        for st in list(reversed(_stacks)):
            try:
                st.close()
            except Exception:
                pass


def _get_nc(dbg=False):
    ph = _phase_limit()
    key = ("nc_dbg" if dbg else "nc") + str(ph)
    if key not in _cache:
        _cache[key] = _build(dbg, ph)
    return _cache[key]


def _qw(w):
    """host-side per-tensor symmetric int8 quant; returns (bf16 ints, f32 scale)."""
    w = np.asarray(w, np.float32)
    s = np.float32(np.abs(w).max()) / np.float32(127.0) + np.float32(1e-8)
    q = np.rint(w / s).astype(BF)
    return q, np.float32(s)


def kernel(inputs, rms1_scale, wq, bq, wk, bk, wv, bv, rel_pos_emb,
           wo, bo, rms2_scale, w1, b1, w2, b2, **_unused):
    global last_results
    f = np.float32
    x = np.ascontiguousarray(np.asarray(inputs, f).reshape(S, DM).T)   # [DM, S]
    xb = x.astype(BF)
    wqq, swq = _qw(wq)
    wkq, swk = _qw(wk)
    wvq, swv = _qw(wv)
    woq, swo = _qw(wo)
    w1q, sw1 = _qw(w1)
    w2q, sw2 = _qw(w2)
    wsc = np.array([[swq, swk, swv, swo, sw1, sw2, 0.0, 0.0]], f)
    rel = np.asarray(rel_pos_emb, f).astype(BF)                        # [65, 64]
    # TT[h, p, m] = rel[clip(m - p - 864, 0, 64), h]; slice cols (896 - kt*128)
    # + q gives biasT[k, q] = rel[clip(q - k + 32)] for k-tile kt.
    idx = np.clip(np.arange(TBL)[None, :] - np.arange(P)[:, None] - 864, 0, 64)
    TTfull = rel[idx]                                                  # [P, TBL, 64]

    in_maps = []
    for c in range(NCORE):
        cs, ce = c * DHL, (c + 1) * DHL
        ms, me = c * MLPL, (c + 1) * MLPL
        in_maps.append({
            "xb": xb,
            "x32": np.ascontiguousarray(x[cs:ce]),
            "sc1": np.asarray(rms1_scale, f),
            "sc2": np.ascontiguousarray(np.asarray(rms2_scale, f)[cs:ce]),
            "wqb": np.ascontiguousarray(wqq[:, cs:ce]),
            "wkb": np.ascontiguousarray(wkq[:, cs:ce]),
            "wvb": np.ascontiguousarray(wvq[:, cs:ce]),
            "bq": np.ascontiguousarray(np.asarray(bq, f)[cs:ce]),
            "bk": np.ascontiguousarray(np.asarray(bk, f)[cs:ce]),
            "bv": np.ascontiguousarray(np.asarray(bv, f)[cs:ce]),
            "tbl": np.ascontiguousarray(
                TTfull[:, :, c * HL:(c + 1) * HL].transpose(2, 0, 1)
                .reshape(HL * P, TBL)),
            "wob": np.ascontiguousarray(woq[cs:ce, :]),
            "bo": np.ascontiguousarray(np.asarray(bo, f)[cs:ce]),
            "w1b": np.ascontiguousarray(w1q[:, ms:me]),
            "b1": np.ascontiguousarray(np.asarray(b1, f)[ms:me]),
            "w2b": np.ascontiguousarray(w2q[ms:me, :]),
            "b2": np.ascontiguousarray(np.asarray(b2, f)[cs:ce]),
            "wsc": wsc,
        })

    dbg = bool(os.environ.get("KERNEL_DEBUG_DUMPS"))
    nc = _get_nc(dbg)
    res = run_bass_kernel_spmd(nc, in_maps, core_ids=list(range(NCORE)))
    last_results = res
    outT = np.concatenate([res.results[c]["out"] for c in range(NCORE)], axis=0)
    return np.ascontiguousarray(outT.T).reshape(1, S, DM).astype(np.float32)



# revision 1
# speedup vs baseline: 1.0482x; 1.0482x over previous
"""Trainium2 Bass kernel for nn_EnhancedTransformerBlock (8-core Megatron TP).

v2 design notes (cost-model-driven rewrite of the working baseline):
- Weights are int8-quantized ON HOST and shipped as bf16 (exact for |v|<=127):
  kills the on-device weight absmax/quantize passes and halves weight HBM
  traffic. Weight scales ride in a tiny f32 consts tensor.
- norm1 is REPLICATED (every core normalizes+quantizes the full residual
  stream locally): no AllGather and no stat AllReduces for layer 1.
- All cross-core max-reductions use small AllGather + local reduce (15us)
  instead of AllReduce (28us).
- Attention computed in transposed orientation: scoresT[k,q] = kk^T qq so the
  softmax denominator is a matmul ones-row and attn weights feed the AV
  matmul without PE transposes. Rel-pos bias is injected into the scores
  PSUM via a diag(1/(sq*sk)) identity matmul (zero elementwise cost).
- WO / MLP reduce-scatters run in bf16; norm2 stats (ssq+colmax) share one
  packed AllGather; x2q crosses cores as int8 in 4 seq-chunked AllGathers
  pipelined under the MLP-up matmul.
"""
import os
from contextlib import ExitStack

import ml_dtypes
import numpy as np

import concourse.bass as bass  # noqa: F401  (engine registry import side effects)
import concourse.mybir as mybir
import concourse.tile as tile
from concourse import bacc, bass_isa
from concourse.bass_utils import run_bass_kernel_spmd
from concourse.masks import make_identity

P = 128
S = 1024
DM = 4096
MLP = 16384
NCORE = 8
HL = 8            # heads per core
DHL = 512         # local qkv/feature shard (HL*64)
MLPL = 2048       # local mlp cols
FT = DM // P      # 32
FTL = DHL // P    # 4
MT = MLPL // P    # 16
TBL = 1920        # rel-pos diag table row length
SF = float(np.sqrt(64.0) * 1024.0 ** 0.25)
MAGIC = 12582912.0  # 1.5*2^23: (x+M)-M == rint(x) for |x|<2^22
NCHUNK = 4        # x2q allgather seq chunks (small first so MLP starts early)
CB = [(0, 128), (128, 384), (384, 640), (640, 1024)]

F32 = mybir.dt.float32
BF16 = mybir.dt.bfloat16
I8 = mybir.dt.int8
AL = mybir.AluOpType
AF = mybir.ActivationFunctionType
AX = mybir.AxisListType
RG = [list(range(NCORE))]
BF = ml_dtypes.bfloat16

_cache = {}
last_results = None


class _Stop(Exception):
    pass


def _bias_src(tbl_dr, h, kt0=0, nkt=4):
    """[P, nkt, S] view of head h's table: (p, kt, q) -> tbl[h*P+p, q + 896 - (kt0+kt)*128].

    With TT[h, p, m] = rel[clip(m - p - 864, 0, 64), h] this reads
    biasT[k0+p, q] = rel[clip(q - (k0+p) + 32)] for k-tile kt (k0 = kt*128).
    """
    src = tbl_dr[:, :].copy()
    v = src.ap
    v[0] = (TBL, P)
    v[1] = (-P, nkt)
    v.append((1, S))
    src.ap = v
    src.offset = h * P * TBL + 896 - kt0 * P
    return src


def _phase_limit():
    v = os.environ.get("KERNEL_PHASE")
    return int(v) if v else 99


def _build(dbg=False, phase=99):
    nc = bacc.Bacc("TRN2", target_bir_lowering=False, debug=False, num_devices=NCORE)
    io = {"_dbg": dbg, "_phase": phase}

    def di(name, shape, dt=F32):
        io[name] = nc.dram_tensor(name, shape, dt, kind="ExternalInput")

    di("xb", [DM, S], BF16)          # full input, transposed, bf16 (norm1 path)
    di("x32", [DHL, S])              # own feature shard fp32 (residual path)
    di("sc1", [DM]); di("sc2", [DHL])
    di("wqb", [DM, DHL], BF16); di("wkb", [DM, DHL], BF16); di("wvb", [DM, DHL], BF16)
    di("bq", [DHL]); di("bk", [DHL]); di("bv", [DHL])
    di("tbl", [HL * P, TBL], BF16)
    di("wob", [DHL, DM], BF16); di("bo", [DHL])
    di("w1b", [DM, MLPL], BF16); di("b1", [MLPL])
    di("w2b", [MLPL, DM], BF16); di("b2", [DHL])
    di("wsc", [1, 8])                # host weight scales: swq swk swv swo sw1 sw2
    io["out"] = nc.dram_tensor("out", [DHL, S], F32, kind="ExternalOutput")
    if dbg:
        io["dbg_xq"] = nc.dram_tensor("dbg_xq", [DM, S], BF16, kind="ExternalOutput")
        io["dbg_q"] = nc.dram_tensor("dbg_q", [DHL, S], BF16, kind="ExternalOutput")
        io["dbg_k"] = nc.dram_tensor("dbg_k", [DHL, S], BF16, kind="ExternalOutput")
        io["dbg_v"] = nc.dram_tensor("dbg_v", [S, DHL], BF16, kind="ExternalOutput")
        io["dbg_ao"] = nc.dram_tensor("dbg_ao", [DHL, S], BF16, kind="ExternalOutput")
        io["dbg_x2"] = nc.dram_tensor("dbg_x2", [DHL, S], F32, kind="ExternalOutput")
        io["dbg_h"] = nc.dram_tensor("dbg_h", [MLPL, S], BF16, kind="ExternalOutput")
        io["dbg_x2q"] = nc.dram_tensor("dbg_x2q", [DM, S], I8, kind="ExternalOutput")
        io["dbg_sc"] = nc.dram_tensor("dbg_sc", [1, 16], F32, kind="ExternalOutput")

    with tile.TileContext(nc) as tc:
        _body(nc, tc, io)
    nc.compile()
    return nc


def _body(nc, tc, io):
    dbg = io["_dbg"]
    _phase = io["_phase"]

    top = ExitStack()
    _stacks = [top]
    const = top.enter_context(tc.tile_pool(name="const", bufs=1))
    dram = top.enter_context(tc.tile_pool(name="dram", bufs=1, space="DRAM"))

    ones_bf = const.tile([P, 1], BF16)
    nc.vector.memset(ones_bf[:, :], 1.0)
    ident = const.tile([P, P], F32)
    make_identity(nc, ident)

    def load_vec(dr, n_tiles, name):
        t = const.tile([P, n_tiles], F32, name=name)
        nc.scalar.dma_start(t[:, :], dr[:].rearrange("(o p) -> p o", p=P))
        return t

    sc1_sb = load_vec(io["sc1"], FT, "sc1_sb")
    sc2_sb = load_vec(io["sc2"], FTL, "sc2_sb")
    bq_sb = load_vec(io["bq"], FTL, "bq_sb")
    bk_sb = load_vec(io["bk"], FTL, "bk_sb")
    bo_sb = load_vec(io["bo"], FTL, "bo_sb")
    b1_sb = load_vec(io["b1"], MT, "b1_sb")
    b2_sb = load_vec(io["b2"], FTL, "b2_sb")
    wsc_sb = const.tile([1, 8], F32, name="wsc_sb")
    nc.scalar.dma_start(wsc_sb[:, :], io["wsc"][:, :])
    bv_row = const.tile([1, DHL], F32, name="bv_row")
    nc.scalar.dma_start(bv_row[:, :], io["bv"][:].unsqueeze(0))
    bv_bc = const.tile([P, DHL], F32, name="bv_bc")
    nc.gpsimd.partition_broadcast(bv_bc[:, :], bv_row[:, :], channels=P)

    def sc11(name):
        return const.tile([1, 1], F32, name=name)

    def bc(src11, name, ch=P):
        t = const.tile([ch, 1], F32, name=name)
        nc.gpsimd.partition_broadcast(t[:, :], src11, channels=ch)
        return t

    def quant_scale(mx11, name):
        """s = mx/127 + 1e-8; returns (s, 1/s) [1,1] tiles."""
        s = sc11(name + "_s")
        nc.vector.tensor_scalar(s[:, :], mx11, 1.0 / 127.0, 1e-8, AL.mult, AL.add)
        inv = sc11(name + "_inv")
        nc.vector.reciprocal(inv[:, :], s[:, :])
        return s, inv

    def smul(a11, b11, name):
        t = sc11(name)
        nc.vector.tensor_tensor(t[:, :], a11, b11, AL.mult)
        return t

    def agmax(vals, tag, q=None):
        """AllGather-based global max of [1,1] APs -> list of [1,1] tiles."""
        n = len(vals)
        loc = const.tile([1, n], F32, name=f"agl_{tag}")
        for i, v in enumerate(vals):
            nc.vector.tensor_copy(loc[:, i:i + 1], v)
        ag_in = dram.tile([1, n], F32, name=f"agi_{tag}")
        ag_out = dram.tile([NCORE, n], F32, addr_space="Shared", name=f"ago_{tag}")
        (q or nc.scalar).dma_start(ag_in[:, :], loc[:, :])
        nc.gpsimd.collective_compute("AllGather", AL.bypass, replica_groups=RG,
                                     ins=[ag_in[:, :].opt()], outs=[ag_out[:, :].opt()])
        g = const.tile([NCORE, n], F32, name=f"agg_{tag}")
        nc.gpsimd.dma_start(g[:, :], ag_out[:, :])
        r = const.tile([NCORE, n], F32, name=f"agr_{tag}")
        nc.gpsimd.partition_all_reduce(r[:, :], g[:, :], channels=NCORE,
                                       reduce_op=bass_isa.ReduceOp.max)
        return [r[:1, i:i + 1] for i in range(n)]

    def _ckpt(n):
        if _phase <= n:
            raise _Stop()

    try:
        # long-lived pools, opened in reverse order of their close points
        ao_cm = ExitStack(); _stacks.append(ao_cm)
        aop = ao_cm.enter_context(tc.tile_pool(name="aop", bufs=1))
        ao_sb = aop.tile([64, HL, S], BF16, name="ao_sb")
        wow_cm = ExitStack(); _stacks.append(wow_cm)
        wow = wow_cm.enter_context(tc.tile_pool(name="wow", bufs=1))
        wo_sb = wow.tile([P, FTL, DM], BF16, name="wo_sb")
        attn_cm = ExitStack(); _stacks.append(attn_cm)
        attnk = attn_cm.enter_context(tc.tile_pool(name="attnk", bufs=1))
        qq_sb = attnk.tile([P, FTL, S], BF16, name="qq_sb")
        kk_sb = attnk.tile([P, FTL, S], BF16, name="kk_sb")
        vq_ext = attnk.tile([P, 8, 8, 66], BF16, name="vq_ext")
        nc.vector.memset(vq_ext[:, :, :, :], 1.0)
        xq_cm = ExitStack(); _stacks.append(xq_cm)
        xqp = xq_cm.enter_context(tc.tile_pool(name="xqp", bufs=1))
        xq_all = xqp.tile([P, FT, S], BF16, name="xq_all")

        # ================= Phase A: norm1, replicated =================
        with tc.tile_pool(name="n1k", bufs=1) as n1k, \
             tc.tile_pool(name="n1t", bufs=2) as n1t, \
             tc.tile_pool(name="n1st", bufs=1) as n1st, \
             tc.tile_pool(name="n1ps", bufs=1, space="PSUM") as n1ps:
            cmax = n1k.tile([P, S], BF16, name="cmax")
            u1_bc = n1k.tile([P, S], BF16, name="u1_bc")
            ssq_ps = n1ps.tile([1, 2, 512], F32, name="ssq_ps")
            for g in range(FT // 4):
                xt = n1t.tile([P, 4, S], BF16, tag="xt", name="xt")
                nc.sync.dma_start(xt[:, :, :],
                                  io["xb"][g * 4 * P:(g + 1) * 4 * P, :]
                                  .rearrange("(o p) f -> p o f", p=P))
                for j in range(4):
                    t = g * 4 + j
                    sq = n1t.tile([P, S], BF16, tag="sq", name="sq")
                    nc.scalar.activation(sq[:, :], xt[:, j, :], AF.Square)
                    for n in range(2):
                        nc.tensor.matmul(ssq_ps[:, n, :], ones_bf[:, :],
                                         sq[:, n * 512:(n + 1) * 512],
                                         start=(t == 0), stop=(t == FT - 1))
                    nc.vector.tensor_scalar(xq_all[:, t, :], xt[:, j, :],
                                            sc1_sb[:, t:t + 1], None, AL.mult)
                    ab = n1t.tile([P, S], BF16, tag="ab", name="ab")
                    nc.scalar.activation(ab[:, :], xq_all[:, t, :], AF.Abs)
                    if t == 0:
                        nc.vector.tensor_copy(cmax[:, :], ab[:, :])
                    else:
                        nc.vector.tensor_tensor(cmax[:, :], cmax[:, :], ab[:, :],
                                                AL.max)

            cmf = n1st.tile([P, S], F32, name="cmf")
            nc.vector.tensor_copy(cmf[:, :], cmax[:, :])
            cmr = n1st.tile([P, S], F32, name="cmr")
            nc.gpsimd.partition_all_reduce(cmr[:, :], cmf[:, :], channels=P,
                                           reduce_op=bass_isa.ReduceOp.max)
            ssq = n1st.tile([1, S], F32, name="ssq")
            nc.scalar.copy(ssq[:, :], ssq_ps[:, :, :].rearrange("p a b -> p (a b)"))
            rstd = n1st.tile([1, S], F32, name="rstd1")
            nc.vector.tensor_scalar(rstd[:, :], ssq[:, :], 1.0 / DM, 1e-6,
                                    AL.mult, AL.add)
            nc.scalar.activation(rstd[:, :], rstd[:, :], AF.Sqrt)
            nc.vector.reciprocal(rstd[:, :], rstd[:, :])
            sxv = n1st.tile([1, S], F32, name="sxv")
            nc.vector.tensor_tensor(sxv[:, :], cmr[:1, :], rstd[:, :], AL.mult)
            mx1 = sc11("mx1")
            nc.vector.tensor_reduce(mx1[:, :], sxv[:, :], AX.X, AL.max)
            sx1, inv_sx1 = quant_scale(mx1[:, :], "sx1")
            u1 = n1st.tile([1, S], BF16, name="u1")
            nc.vector.tensor_scalar(u1[:, :], rstd[:, :], inv_sx1[:, :], None, AL.mult)
            nc.gpsimd.partition_broadcast(u1_bc[:, :], u1[:, :], channels=P)
            # in-place quantize: xq_all currently holds xs = x*sc1
            for t in range(FT):
                nc.vector.tensor_tensor(xq_all[:, t, :], xq_all[:, t, :],
                                        u1_bc[:, :], AL.mult)
                nc.vector.tensor_scalar(xq_all[:, t, :], xq_all[:, t, :],
                                        MAGIC, MAGIC, AL.add, AL.subtract)
        if dbg:
            nc.sync.dma_start(io["dbg_xq"][:, :].rearrange("(o p) f -> p o f", p=P),
                              xq_all[:, :, :])
        _ckpt(1)

        # ================= Phase B: QKV projections =================
        nc.sync.dma_start(wo_sb[:, :, :],
                          io["wob"][:, :].rearrange("(g p) c -> p g c", p=P))
        aq_bc = bc(smul(sx1[:, :], wsc_sb[:, 0:1], "aq")[:, :], "aq_bc")
        ak_bc = bc(smul(sx1[:, :], wsc_sb[:, 1:2], "ak")[:, :], "ak_bc")
        av_bc = bc(smul(sx1[:, :], wsc_sb[:, 2:3], "av")[:, :], "av_bc")

        vv_cm = ExitStack(); _stacks.append(vv_cm)
        vvf = vv_cm.enter_context(tc.tile_pool(name="vvf", bufs=1))
        v_bf = vvf.tile([P, 8, DHL], BF16, name="v_bf")
        qk_cm = ExitStack(); _stacks.append(qk_cm)
        qkf = qk_cm.enter_context(tc.tile_pool(name="qkf", bufs=1))
        q_bf = qkf.tile([P, FTL, S], BF16, name="q_bf")
        k_bf = qkf.tile([P, FTL, S], BF16, name="k_bf")

        qmaxs = const.tile([P, 3], F32, name="qkv_max")
        with tc.tile_pool(name="wld", bufs=2) as wldp, \
             tc.tile_pool(name="qkev", bufs=3) as qev, \
             tc.tile_pool(name="qkvps", bufs=1, space="PSUM") as qkv_ps:
            for wi, (which, w_dr, alpha, bias_sb, dest) in enumerate((
                    ("q", io["wqb"], aq_bc, bq_sb, q_bf),
                    ("k", io["wkb"], ak_bc, bk_sb, k_bf))):
                pss = [qkv_ps.tile([P, 512], F32, tag=f"ps{i}", name=f"ps_{which}{i}")
                       for i in range(8)]
                for k0 in range(0, FT, 4):
                    wb = wldp.tile([P, 4, DHL], BF16, tag="wqk", name=f"w_{which}")
                    nc.sync.dma_start(wb[:, :, :],
                                      w_dr[k0 * P:(k0 + 4) * P, :]
                                      .rearrange("(g p) c -> p g c", p=P))
                    for g in range(4):
                        k = k0 + g
                        for m in range(FTL):
                            for n in range(2):
                                nc.tensor.matmul(pss[m * 2 + n][:, :],
                                                 wb[:, g, m * P:(m + 1) * P],
                                                 xq_all[:, k, n * 512:(n + 1) * 512],
                                                 start=(k == 0), stop=(k == FT - 1))
                red = qev.tile([P, FTL, 2], F32, tag=f"red{which}", name=f"red_{which}")
                for m in range(FTL):
                    for n in range(2):
                        nc.scalar.activation(dest[:, m, n * 512:(n + 1) * 512],
                                             pss[m * 2 + n][:, :], AF.Identity,
                                             bias=bias_sb[:, m:m + 1],
                                             scale=alpha[:, :1])
                        nc.vector.tensor_reduce(red[:, m, n:n + 1],
                                                dest[:, m, n * 512:(n + 1) * 512],
                                                AX.X, AL.max, apply_absolute_value=True)
                nc.vector.tensor_reduce(qmaxs[:, wi:wi + 1], red[:, :, :], AX.XY, AL.max)

            # launch q/k max allgather while V matmuls run
            parq = const.tile([P, 2], F32, name="parqk")
            nc.gpsimd.partition_all_reduce(parq[:, :], qmaxs[:, 0:2], channels=P,
                                           reduce_op=bass_isa.ReduceOp.max)
            gq, gk = agmax([parq[:1, 0:1], parq[:1, 1:2]], "qk")
            sq_s, invq = quant_scale(gq, "sq")
            sk_s, invk = quant_scale(gk, "sk")
            invq_bc, invk_bc = bc(invq[:, :], "invq_bc"), bc(invk[:, :], "invk_bc")
            sqk = smul(sq_s[:, :], sk_s[:, :], "sqk")
            alpha = sc11("alpha")
            nc.vector.tensor_scalar(alpha[:, :], sqk[:, :], 1.0 / SF, None, AL.mult)
            alpha_bc = bc(alpha[:, :], "alpha_bc")
            inv_sqk = sc11("inv_sqk")
            nc.vector.reciprocal(inv_sqk[:, :], sqk[:, :])
            inv_sqk_bc = bc(inv_sqk[:, :], "inv_sqk_bc")
            identc = const.tile([P, P], BF16, name="identc")
            nc.vector.tensor_scalar(identc[:, :], ident[:, :], inv_sqk_bc[:, :1],
                                    None, AL.mult)

            pss_v = [qkv_ps.tile([P, 512], F32, tag=f"ps{i}", name=f"ps_v{i}")
                     for i in range(8)]
            for k0 in range(0, FT, 4):
                wb = wldp.tile([P, 4, DHL], BF16, tag="wqk", name="w_v")
                nc.sync.dma_start(wb[:, :, :],
                                  io["wvb"][k0 * P:(k0 + 4) * P, :]
                                  .rearrange("(g p) c -> p g c", p=P))
                for g in range(4):
                    k = k0 + g
                    for m in range(8):
                        nc.tensor.matmul(pss_v[m][:, :],
                                         xq_all[:, k, m * P:(m + 1) * P],
                                         wb[:, g, :],
                                         start=(k == 0), stop=(k == FT - 1))
            vred = qev.tile([P, 8], F32, tag="vred", name="vred")
            for m in range(8):
                ev = qev.tile([P, DHL], F32, tag="vev", name="vev")
                nc.scalar.mul(ev[:, :], pss_v[m][:, :], av_bc[:, :1])
                nc.vector.tensor_tensor(v_bf[:, m, :], ev[:, :], bv_bc[:, :], AL.add)
                nc.vector.tensor_reduce(vred[:, m:m + 1], v_bf[:, m, :], AX.X,
                                        AL.max, apply_absolute_value=True)
            nc.vector.tensor_reduce(qmaxs[:, 2:3], vred[:, :], AX.X, AL.max)
            parv = const.tile([P, 1], F32, name="parv")
            nc.gpsimd.partition_all_reduce(parv[:, :], qmaxs[:, 2:3], channels=P,
                                           reduce_op=bass_isa.ReduceOp.max)
            (gv,) = agmax([parv[:1, :]], "v")
        if dbg:
            nc.sync.dma_start(io["dbg_q"][:, :].rearrange("(o p) f -> p o f", p=P),
                              q_bf[:, :, :])
            nc.sync.dma_start(io["dbg_k"][:, :].rearrange("(o p) f -> p o f", p=P),
                              k_bf[:, :, :])
            nc.sync.dma_start(io["dbg_v"][:, :].rearrange("(o p) f -> p o f", p=P),
                              v_bf[:, :, :])
        _ckpt(2)

        # ================= Phase C: quantize q/k/v =================
        with tc.tile_pool(name="qknt", bufs=2) as qknt:
            for t in range(FTL):
                for src, dst, ibc in ((q_bf, qq_sb, invq_bc), (k_bf, kk_sb, invk_bc)):
                    tmp = qknt.tile([P, S], BF16, tag="qkq", name="qkq")
                    nc.scalar.mul(tmp[:, :], src[:, t, :], ibc[:, :1])
                    nc.vector.tensor_scalar(dst[:, t, :], tmp[:, :], MAGIC, MAGIC,
                                            AL.add, AL.subtract)
        qk_cm.close()
        _stacks.remove(qk_cm)
        with tc.tile_pool(name="vqnt", bufs=1) as vqnt:
            sv_s, invv = quant_scale(gv, "sv")
            invv_bc = bc(invv[:, :], "invv_bc")
            tmp = vqnt.tile([P, 8, DHL], BF16, tag="vq", name="vqt")
            nc.vector.tensor_scalar(tmp[:, :, :], v_bf[:, :, :], invv_bc[:, :1],
                                    None, AL.mult)
            nc.vector.tensor_scalar(
                vq_ext[:, :, :, 0:64],
                tmp[:, :, :].rearrange("p m (h d) -> p m h d", h=8),
                MAGIC, MAGIC, AL.add, AL.subtract)
        vv_cm.close()
        _stacks.remove(vv_cm)
        xq_cm.close()
        _stacks.remove(xq_cm)
        _ckpt(3)

        # ================= Phase D: attention =================
        aomax = const.tile([64, HL], F32, name="aomax")
        with tc.tile_pool(name="att", bufs=2) as att, \
             tc.tile_pool(name="attb", bufs=2) as attb, \
             tc.tile_pool(name="scps", bufs=2, space="PSUM") as scps, \
             tc.tile_pool(name="avps", bufs=2, space="PSUM") as avps:
            for h in range(HL):
                pb = 64 * (h % 2)
                ht = h // 2
                bias_t = att.tile([P, 8, S], BF16, tag="bias", name="bias")
                nc.scalar.dma_start(bias_t[:, 0:4, :], _bias_src(io["tbl"], h, 0))
                nc.scalar.dma_start(bias_t[:, 4:8, :], _bias_src(io["tbl"], h, 4))
                attnT = att.tile([P, 8, S], BF16, tag="attnT", name="attnT")
                avp = avps.tile([P, 2, 512], F32, tag="avp", name="avp")
                for kt in range(8):
                    ps = scps.tile([P, 2, 512], F32, tag="sc", name="sc_ps")
                    for n in range(2):
                        nc.tensor.matmul(ps[:, n, :],
                                         kk_sb[pb:pb + 64, ht, kt * P:(kt + 1) * P],
                                         qq_sb[pb:pb + 64, ht, n * 512:(n + 1) * 512],
                                         start=True, stop=False)
                        nc.tensor.matmul(ps[:, n, :], identc[:, :],
                                         bias_t[:, kt, n * 512:(n + 1) * 512],
                                         start=False, stop=True)
                    nc.scalar.activation(attnT[:, kt, :],
                                         ps[:, :, :].rearrange("p a b -> p (a b)"),
                                         AF.Exp, scale=alpha_bc[:, :1])
                    for n in range(2):
                        nc.tensor.matmul(avp[:65, n, :],
                                         vq_ext[:, kt, h, 0:65],
                                         attnT[:, kt, n * 512:(n + 1) * 512],
                                         start=(kt == 0), stop=(kt == 7))
                den = attb.tile([1, S], F32, tag="den", name="den")
                nc.vector.tensor_scalar(den[:, :],
                                        avp[64:65, :, :].rearrange("p a b -> p (a b)"),
                                        1e-6, None, AL.add)
                nc.vector.reciprocal(den[:, :], den[:, :])
                rbc = attb.tile([64, S], F32, tag="rbc", name="rbc")
                nc.gpsimd.partition_broadcast(rbc[:, :], den[:, :], channels=64)
                for n in range(2):
                    nc.vector.tensor_tensor(ao_sb[:, h, n * 512:(n + 1) * 512],
                                            avp[:64, n, :],
                                            rbc[:, n * 512:(n + 1) * 512], AL.mult)
                nc.vector.tensor_reduce(aomax[:, h:h + 1], ao_sb[:, h, :], AX.X,
                                        AL.max, apply_absolute_value=True)
        attn_cm.close()
        _stacks.remove(attn_cm)
        if dbg:
            nc.sync.dma_start(
                io["dbg_ao"][:, :].rearrange("(h d) f -> d h f", h=HL), ao_sb[:, :, :])

        aored = const.tile([64, 1], F32, name="aored")
        nc.vector.tensor_reduce(aored[:, :], aomax[:, :], AX.X, AL.max)
        aopar = const.tile([64, 1], F32, name="aopar")
        nc.gpsimd.partition_all_reduce(aopar[:, :], aored[:, :], channels=64,
                                       reduce_op=bass_isa.ReduceOp.max)
        (graw,) = agmax([aopar[:1, :]], "ao", q=nc.sync)
        _ckpt(4)

        # ================= Phase E: quantize ao, WO matmul, RS =================
        s_ao = sc11("s_ao")
        nc.vector.tensor_tensor(s_ao[:, :], sv_s[:, :], graw, AL.mult)
        nc.vector.tensor_scalar(s_ao[:, :], s_ao[:, :], 1.0 / 127.0, 1e-8,
                                AL.mult, AL.add)
        inv_sao = sc11("inv_sao")
        nc.vector.reciprocal(inv_sao[:, :], s_ao[:, :])
        m_ao_bc = bc(smul(sv_s[:, :], inv_sao[:, :], "m_ao")[:, :], "m_ao_bc", ch=64)
        a_wo_bc = bc(smul(s_ao[:, :], wsc_sb[:, 3:4], "a_wo")[:, :], "a_wo_bc")

        wo_cm = ExitStack(); _stacks.append(wo_cm)
        wop = wo_cm.enter_context(tc.tile_pool(name="wop", bufs=1))
        aoq_sb = wop.tile([P, FTL, S], BF16, name="aoq_sb")
        x2_cm = ExitStack(); _stacks.append(x2_cm)
        x2p = x2_cm.enter_context(tc.tile_pool(name="x2p", bufs=1))
        x2_sb = x2p.tile([P, FTL, S], F32, name="x2_sb")
        x2q_cm = ExitStack(); _stacks.append(x2q_cm)
        x2qp = x2q_cm.enter_context(tc.tile_pool(name="x2qp", bufs=1))
        x2q_i8 = x2qp.tile([P, FTL, S], I8, name="x2q_i8")
        x32_cm = ExitStack(); _stacks.append(x32_cm)
        x32p = x32_cm.enter_context(tc.tile_pool(name="x32p", bufs=1))
        x32_sb = x32p.tile([P, FTL, S], F32, name="x32_sb")
        nc.sync.dma_start(x32_sb[:, :, :],
                          io["x32"][:, :].rearrange("(o p) f -> p o f", p=P))

        with tc.tile_pool(name="aoqt", bufs=2) as aoqt:
            for h in range(HL):
                tmp = aoqt.tile([64, S], BF16, tag="aoq", name="aoqh")
                nc.scalar.mul(tmp[:, :], ao_sb[:, h, :], m_ao_bc[:, :1])
                if h % 2 == 0:
                    nc.vector.tensor_scalar(aoq_sb[0:64, h // 2, :], tmp[:, :],
                                            MAGIC, MAGIC, AL.add, AL.subtract)
                else:
                    tmp2 = aoqt.tile([64, S], BF16, tag="aoq2", name="aoqh2")
                    nc.vector.tensor_scalar(tmp2[:, :], tmp[:, :],
                                            MAGIC, MAGIC, AL.add, AL.subtract)
                    nc.gpsimd.dma_start(aoq_sb[64:128, h // 2, :], tmp2[:, :])

        aout_d = [dram.tile([DM, 512], BF16, name=f"aout{n}") for n in range(2)]
        rs_d = [dram.tile([DHL, 512], BF16, name=f"aors{n}") for n in range(2)]
        with tc.tile_pool(name="woev", bufs=3) as woev, \
             tc.tile_pool(name="wops", bufs=2, space="PSUM") as wops:
            for n in range(2):
                for mg in range(8):
                    ps = wops.tile([P, 4, 512], F32, tag="wops", name="wo_ps")
                    for k in range(FTL):
                        for mi in range(4):
                            m = mg * 4 + mi
                            nc.tensor.matmul(ps[:, mi, :],
                                             wo_sb[:, k, m * P:(m + 1) * P],
                                             aoq_sb[:, k, n * 512:(n + 1) * 512],
                                             start=(k == 0), stop=(k == FTL - 1))
                    ev = woev.tile([P, 4, 512], BF16, tag="woev", name="wo_ev")
                    nc.scalar.mul(ev[:, :, :], ps[:, :, :], a_wo_bc[:, :1])
                    nc.sync.dma_start(
                        aout_d[n][mg * 4 * P:(mg + 1) * 4 * P, :]
                        .rearrange("(g p) c -> p g c", p=P), ev[:, :, :])
                nc.gpsimd.collective_compute("ReduceScatter", AL.add, replica_groups=RG,
                                             ins=[aout_d[n][:, :].opt()],
                                             outs=[rs_d[n][:, :].opt()])

        # x2 build fused with norm2 stats (per seq-half, right behind each RS)
        with tc.tile_pool(name="x2t", bufs=2) as x2t, \
             tc.tile_pool(name="n2t", bufs=2) as n2t, \
             tc.tile_pool(name="n2ps", bufs=1, space="PSUM") as n2ps:
            ssq2_ps = n2ps.tile([1, 2, 512], F32, name="ssq2_ps")
            cm2 = n2t.tile([P, S], F32, tag="cm2", name="cm2")
            for n in range(2):
                h0, h1 = n * 512, (n + 1) * 512
                rst = x2t.tile([P, FTL, 512], BF16, tag="rst", name="rst")
                nc.sync.dma_start(rst[:, :, :],
                                  rs_d[n][:, :].rearrange("(o p) f -> p o f", p=P))
                for t in range(FTL):
                    tmp = x2t.tile([P, 512], F32, tag="x2tmp", name="x2tmp")
                    nc.vector.tensor_scalar(tmp[:, :], rst[:, t, :],
                                            bo_sb[:, t:t + 1], None, AL.add)
                    nc.vector.tensor_tensor(x2_sb[:, t, h0:h1], tmp[:, :],
                                            x32_sb[:, t, h0:h1], AL.add)
                    sq = n2t.tile([P, 512], BF16, tag="sq2", name="sq2")
                    nc.scalar.activation(sq[:, :], x2_sb[:, t, h0:h1], AF.Square)
                    nc.tensor.matmul(ssq2_ps[:, n, :], ones_bf[:, :], sq[:, :],
                                     start=(t == 0), stop=(t == FTL - 1))
                    xs2 = n2t.tile([P, 512], F32, tag="xs2", name="xs2")
                    nc.vector.tensor_scalar(xs2[:, :], x2_sb[:, t, h0:h1],
                                            sc2_sb[:, t:t + 1], None, AL.mult)
                    ab2 = n2t.tile([P, 512], F32, tag="ab2", name="ab2")
                    nc.scalar.activation(ab2[:, :], xs2[:, :], AF.Abs)
                    if t == 0:
                        nc.vector.tensor_copy(cm2[:, h0:h1], ab2[:, :])
                    else:
                        nc.vector.tensor_tensor(cm2[:, h0:h1], cm2[:, h0:h1],
                                                ab2[:, :], AL.max)
            cm2r_x = x2qp.tile([P, S], F32, name="cm2r_x")
            nc.gpsimd.partition_all_reduce(cm2r_x[:, :], cm2[:, :], channels=P,
                                           reduce_op=bass_isa.ReduceOp.max)
            ssq2_row = x2qp.tile([1, S], F32, name="ssq2_row")
            nc.scalar.copy(ssq2_row[:, :],
                           ssq2_ps[:, :, :].rearrange("p a b -> p (a b)"))
        x32_cm.close()
        _stacks.remove(x32_cm)
        if dbg:
            nc.sync.dma_start(io["dbg_x2"][:, :].rearrange("(o p) f -> p o f", p=P),
                              x2_sb[:, :, :])
        _ckpt(5)

        # ================= Phase F: norm2 reduce + quantize + chunked AG ====
        agx_in = [dram.tile([DHL, c1 - c0], I8, name=f"agx_in{c}")
                  for c, (c0, c1) in enumerate(CB)]
        agx_out = [dram.tile([DM, c1 - c0], I8, addr_space="Shared", name=f"agx_out{c}")
                   for c, (c0, c1) in enumerate(CB)]
        with tc.tile_pool(name="n2b", bufs=2) as n2t:
            pack = n2t.tile([1, 2 * S], F32, tag="pack", name="pack")
            nc.vector.tensor_copy(pack[:, 0:S], ssq2_row[:, :])
            nc.vector.tensor_copy(pack[:, S:2 * S], cm2r_x[:1, :])
            st_in = dram.tile([1, 2 * S], F32, name="st_in")
            st_out = dram.tile([NCORE, 2 * S], F32, addr_space="Shared", name="st_out")
            nc.scalar.dma_start(st_in[:, :], pack[:, :])
            nc.gpsimd.collective_compute("AllGather", AL.bypass, replica_groups=RG,
                                         ins=[st_in[:, :].opt()],
                                         outs=[st_out[:, :].opt()])
            stg = n2t.tile([NCORE, 2 * S], F32, tag="stg", name="stg")
            nc.gpsimd.dma_start(stg[:, :], st_out[:, :])
            ssq2g = n2t.tile([NCORE, S], F32, tag="ssq2g", name="ssq2g")
            nc.gpsimd.partition_all_reduce(ssq2g[:, :], stg[:, 0:S], channels=NCORE,
                                           reduce_op=bass_isa.ReduceOp.add)
            cm2g = n2t.tile([NCORE, S], F32, tag="cm2g", name="cm2g")
            nc.gpsimd.partition_all_reduce(cm2g[:, :], stg[:, S:2 * S], channels=NCORE,
                                           reduce_op=bass_isa.ReduceOp.max)
            rstd2 = n2t.tile([1, S], F32, tag="rstd2", name="rstd2")
            nc.vector.tensor_scalar(rstd2[:, :], ssq2g[:1, :], 1.0 / DM, 1e-6,
                                    AL.mult, AL.add)
            nc.scalar.activation(rstd2[:, :], rstd2[:, :], AF.Sqrt)
            nc.vector.reciprocal(rstd2[:, :], rstd2[:, :])
            sxv2 = n2t.tile([1, S], F32, tag="sxv2", name="sxv2")
            nc.vector.tensor_tensor(sxv2[:, :], cm2g[:1, :], rstd2[:, :], AL.mult)
            mx2 = sc11("mx2")
            nc.vector.tensor_reduce(mx2[:, :], sxv2[:, :], AX.X, AL.max)
            sx2, inv_sx2 = quant_scale(mx2[:, :], "sx2")
            u2 = n2t.tile([1, S], F32, tag="u2", name="u2")
            nc.vector.tensor_scalar(u2[:, :], rstd2[:, :], inv_sx2[:, :], None, AL.mult)
            u2_bc = n2t.tile([P, S], F32, tag="u2bc", name="u2_bc")
            nc.gpsimd.partition_broadcast(u2_bc[:, :], u2[:, :], channels=P)
            # quantize + allgather chunk by chunk so the first AG fires early
            for c, (c0, c1) in enumerate(CB):
                for t in range(FTL):
                    xs2 = n2t.tile([P, c1 - c0], F32, tag="xs2b", name="xs2b")
                    nc.vector.tensor_scalar(xs2[:, :], x2_sb[:, t, c0:c1],
                                            sc2_sb[:, t:t + 1], None, AL.mult)
                    xnq = n2t.tile([P, c1 - c0], F32, tag="xnq", name="xnq")
                    nc.vector.tensor_tensor(xnq[:, :], xs2[:, :],
                                            u2_bc[:, c0:c1], AL.mult)
                    xqb = n2t.tile([P, c1 - c0], BF16, tag="xqb", name="xqb")
                    nc.vector.tensor_scalar(xqb[:, :], xnq[:, :], MAGIC, MAGIC,
                                            AL.add, AL.subtract)
                    nc.vector.tensor_copy(x2q_i8[:, t, c0:c1], xqb[:, :])
                nc.scalar.dma_start(
                    agx_in[c][:, :].rearrange("(o p) f -> p o f", p=P),
                    x2q_i8[:, :, c0:c1])
                nc.gpsimd.collective_compute("AllGather", AL.bypass, replica_groups=RG,
                                             ins=[agx_in[c][:, :].opt()],
                                             outs=[agx_out[c][:, :].opt()])
        _ckpt(6)
        x2q_cm.close()
        _stacks.remove(x2q_cm)
        # spill x2 (final residual input) to free SBUF for the MLP weights.
        # On the ACT queue: on SP it head-of-line blocks the w1 prefetch.
        x2_dram = dram.tile([DHL, S], F32, name="x2_dram")
        nc.scalar.dma_start(x2_dram[:, :].rearrange("(o p) f -> p o f", p=P),
                            x2_sb[:, :, :])
        x2_cm.close()
        _stacks.remove(x2_cm)
        wo_cm.close()
        _stacks.remove(wo_cm)
        wow_cm.close()
        _stacks.remove(wow_cm)
        ao_cm.close()
        _stacks.remove(ao_cm)

        # ================= Phase G: MLP up =================
        a1_bc = bc(smul(sx2[:, :], wsc_sb[:, 4:5], "a1")[:, :], "a1_bc")
        hmax = const.tile([P, MT], F32, name="hmax")

        h_cm = ExitStack(); _stacks.append(h_cm)
        hp = h_cm.enter_context(tc.tile_pool(name="hp", bufs=1))
        h_sb = hp.tile([P, MT, S], BF16, name="h_sb")
        w1_cm = ExitStack(); _stacks.append(w1_cm)
        w1p = w1_cm.enter_context(tc.tile_pool(name="w1p", bufs=1))
        w1_sb = w1p.tile([P, FT, MLPL], BF16, name="w1_sb")
        # groups 0-1 land on addresses WAR-pinned by x2 until its last read;
        # load them last and rotate the k-loop so they are consumed last too
        for g in (3, 4, 5, 6, 7):
            k0 = g * 4
            nc.sync.dma_start(w1_sb[:, k0:k0 + 4, :],
                              io["w1b"][k0 * P:(k0 + 4) * P, :]
                              .rearrange("(g p) c -> p g c", p=P))
        # groups 0-2 sit on addresses pinned until ~norm2-quant; small pieces
        # so the x2q staging transfer is not stuck behind a long backlog
        for g in (0, 1, 2):
            for half in range(2):
                k0 = g * 4 + half * 2
                nc.sync.dma_start(w1_sb[:, k0:k0 + 2, :],
                                  io["w1b"][k0 * P:(k0 + 2) * P, :]
                                  .rearrange("(g p) c -> p g c", p=P))

        with tc.tile_pool(name="m1s", bufs=1) as m1s, \
             tc.tile_pool(name="m1ps", bufs=2, space="PSUM") as m1ps:
            for c, (c0, c1) in enumerate(CB):
                cw = c1 - c0
                xgb = m1s.tile([P, FT, 256 if c % 2 == 0 else 384], BF16,
                               tag="xgbA" if c % 2 == 0 else "xgbB", name="xgb")
                nc.gpsimd.dma_start(
                    xgb[:, :, 0:cw],
                    agx_out[c][:, :].rearrange("(o p) f -> p o f", p=P))
                for mg in range(4):
                    # [P, 4, 512] so each mi region owns a full 2KB PSUM bank:
                    # matmul start=True clears at bank granularity.
                    ps = m1ps.tile([P, 4, 512], F32, tag="m1ps", name="m1_ps")
                    for ki in range(FT):
                        k = (ki + 12) % FT
                        for mi in range(4):
                            m = mg * 4 + mi
                            nc.tensor.matmul(ps[:, mi, 0:cw],
                                             w1_sb[:, k, m * P:(m + 1) * P],
                                             xgb[:, k, 0:cw],
                                             start=(ki == 0), stop=(ki == FT - 1))
                    for mi in range(4):
                        m = mg * 4 + mi
                        nc.scalar.activation(h_sb[:, m, c0:c1], ps[:, mi, 0:cw],
                                             AF.Gelu_apprx_tanh,
                                             bias=b1_sb[:, m:m + 1], scale=a1_bc[:, :1])
                    nc.vector.tensor_reduce(
                        hmax[:, c * 4 + mg:c * 4 + mg + 1],
                        h_sb[:, mg * 4:(mg + 1) * 4, c0:c1], AX.XY, AL.max,
                        apply_absolute_value=True)
        w1_cm.close()
        _stacks.remove(w1_cm)
        if dbg:
            nc.sync.dma_start(io["dbg_h"][:, :].rearrange("(o p) f -> p o f", p=P),
                              h_sb[:, :, :])

        hred = const.tile([P, 1], F32, name="hred")
        nc.vector.tensor_reduce(hred[:, :], hmax[:, :], AX.X, AL.max)
        hpar = const.tile([P, 1], F32, name="hpar")
        nc.gpsimd.partition_all_reduce(hpar[:, :], hred[:, :], channels=P,
                                       reduce_op=bass_isa.ReduceOp.max)
        (gh,) = agmax([hpar[:1, :]], "h", q=nc.sync)
        _ckpt(7)

        # ================= Phase H: MLP down =================
        sh_s, invh = quant_scale(gh, "sh")
        invh_bc = bc(invh[:, :], "invh_bc")
        a2_bc = bc(smul(sh_s[:, :], wsc_sb[:, 5:6], "a2")[:, :], "a2_bc")

        w2_cm = ExitStack(); _stacks.append(w2_cm)
        w2p = w2_cm.enter_context(tc.tile_pool(name="w2p", bufs=1))
        w2_sb = w2p.tile([P, MT, DM], BF16, name="w2_sb")
        for mg in range(8):
            nc.sync.dma_start(
                w2_sb[:, :, mg * 512:(mg + 1) * 512],
                io["w2b"][:, mg * 512:(mg + 1) * 512]
                .rearrange("(g p) c -> p g c", p=P))
        # quantize h in place (values become the int8 grid in bf16);
        # spread across DVE and gpsimd so the serial chain is shorter
        for g in range(MT // 4):
            sl = h_sb[:, g * 4:(g + 1) * 4, :]
            eng = nc.gpsimd if g == 3 else nc.vector
            eng.tensor_scalar(sl, sl, invh_bc[:, :1], None, AL.mult)
            eng.tensor_scalar(sl, sl, MAGIC, MAGIC, AL.add, AL.subtract)
        hq_sb = h_sb

        y_d = [dram.tile([DM, 256], BF16, name=f"y{n}") for n in range(4)]
        yrs_d = [dram.tile([DHL, 256], BF16, name=f"yrs{n}") for n in range(4)]
        with tc.tile_pool(name="m2ev", bufs=3) as m2ev, \
             tc.tile_pool(name="m2ps", bufs=2, space="PSUM") as m2ps:
            for n in range(4):
                for mg in range(8):
                    ps = m2ps.tile([P, 4, 512], F32, tag="m2ps", name="m2_ps")
                    for k in range(MT):
                        for mi in range(4):
                            m = mg * 4 + mi
                            nc.tensor.matmul(ps[:, mi, 0:256],
                                             w2_sb[:, k, m * P:(m + 1) * P],
                                             hq_sb[:, k, n * 256:(n + 1) * 256],
                                             start=(k == 0), stop=(k == MT - 1))
                    ev = m2ev.tile([P, 4, 256], BF16, tag="m2ev", name="m2_ev")
                    nc.scalar.mul(ev[:, :, :], ps[:, :, 0:256], a2_bc[:, :1])
                    nc.sync.dma_start(
                        y_d[n][mg * 4 * P:(mg + 1) * 4 * P, :]
                        .rearrange("(g p) c -> p g c", p=P), ev[:, :, :])
                nc.gpsimd.collective_compute("ReduceScatter", AL.add, replica_groups=RG,
                                             ins=[y_d[n][:, :].opt()],
                                             outs=[yrs_d[n][:, :].opt()])
        w2_cm.close()
        _stacks.remove(w2_cm)
        h_cm.close()
        _stacks.remove(h_cm)
        _ckpt(8)

        if dbg:
            scs = [sx1[:, :], sq_s[:, :], sk_s[:, :], sv_s[:, :], s_ao[:, :],
                   sx2[:, :], sh_s[:, :], alpha[:, :], inv_sqk[:, :], graw, gh,
                   gq, gk, gv, mx1[:, :], mx2[:, :]]
            scv = const.tile([1, 16], F32, name="dbg_scv")
            for i, s in enumerate(scs):
                nc.vector.tensor_copy(scv[:, i:i + 1], s)
            nc.sync.dma_start(io["dbg_sc"][:, :], scv[:, :])

        with tc.tile_pool(name="fint", bufs=2) as fint:
            for n in range(4):
                n0, n1 = n * 256, (n + 1) * 256
                yt = fint.tile([P, FTL, 256], BF16, tag="yrst", name="yrst")
                nc.sync.dma_start(yt[:, :, :],
                                  yrs_d[n][:, :].rearrange("(o p) f -> p o f", p=P))
                x2r = fint.tile([P, FTL, 256], F32, tag="x2r", name="x2r")
                nc.sync.dma_start(x2r[:, :, :],
                                  x2_dram[:, n0:n1]
                                  .rearrange("(o p) f -> p o f", p=P))
                out_t = fint.tile([P, FTL, 256], F32, tag="outt", name="outt")
                for t in range(FTL):
                    tmp = fint.tile([P, 256], F32, tag="fin", name="fin")
                    nc.vector.tensor_scalar(tmp[:, :], yt[:, t, :],
                                            b2_sb[:, t:t + 1], None, AL.add)
                    nc.vector.tensor_tensor(out_t[:, t, :], tmp[:, :],
                                            x2r[:, t, :], AL.add)
                nc.sync.dma_start(
                    io["out"][:, n0:n1]
                    .rearrange("(o p) f -> p o f", p=P), out_t[:, :, :])

    except _Stop:
        pass
    finally:
        for st in list(reversed(_stacks)):
            try:
                st.close()
            except Exception:
                pass


def _get_nc(dbg=False):
    ph = _phase_limit()
    key = ("nc_dbg" if dbg else "nc") + str(ph)
    if key not in _cache:
        _cache[key] = _build(dbg, ph)
    return _cache[key]


def _qw(w):
    """host-side per-tensor symmetric int8 quant; returns (bf16 ints, f32 scale)."""
    w = np.asarray(w, np.float32)
    s = np.float32(np.abs(w).max()) / np.float32(127.0) + np.float32(1e-8)
    q = np.rint(w / s).astype(BF)
    return q, np.float32(s)


def kernel(inputs, rms1_scale, wq, bq, wk, bk, wv, bv, rel_pos_emb,
           wo, bo, rms2_scale, w1, b1, w2, b2, **_unused):
    global last_results
    f = np.float32
    x = np.ascontiguousarray(np.asarray(inputs, f).reshape(S, DM).T)   # [DM, S]
    xb = x.astype(BF)
    wqq, swq = _qw(wq)
    wkq, swk = _qw(wk)
    wvq, swv = _qw(wv)
    woq, swo = _qw(wo)
    w1q, sw1 = _qw(w1)
    w2q, sw2 = _qw(w2)
    wsc = np.array([[swq, swk, swv, swo, sw1, sw2, 0.0, 0.0]], f)
    rel = np.asarray(rel_pos_emb, f).astype(BF)                        # [65, 64]
    # TT[h, p, m] = rel[clip(m - p - 864, 0, 64), h]; slice cols (896 - kt*128)
    # + q gives biasT[k, q] = rel[clip(q - k + 32)] for k-tile kt.
    idx = np.clip(np.arange(TBL)[None, :] - np.arange(P)[:, None] - 864, 0, 64)
    TTfull = rel[idx]                                                  # [P, TBL, 64]

    in_maps = []
    for c in range(NCORE):
        cs, ce = c * DHL, (c + 1) * DHL
        ms, me = c * MLPL, (c + 1) * MLPL
        in_maps.append({
            "xb": xb,
            "x32": np.ascontiguousarray(x[cs:ce]),
            "sc1": np.asarray(rms1_scale, f),
            "sc2": np.ascontiguousarray(np.asarray(rms2_scale, f)[cs:ce]),
            "wqb": np.ascontiguousarray(wqq[:, cs:ce]),
            "wkb": np.ascontiguousarray(wkq[:, cs:ce]),
            "wvb": np.ascontiguousarray(wvq[:, cs:ce]),
            "bq": np.ascontiguousarray(np.asarray(bq, f)[cs:ce]),
            "bk": np.ascontiguousarray(np.asarray(bk, f)[cs:ce]),
            "bv": np.ascontiguousarray(np.asarray(bv, f)[cs:ce]),
            "tbl": np.ascontiguousarray(
                TTfull[:, :, c * HL:(c + 1) * HL].transpose(2, 0, 1)
                .reshape(HL * P, TBL)),
            "wob": np.ascontiguousarray(woq[cs:ce, :]),
            "bo": np.ascontiguousarray(np.asarray(bo, f)[cs:ce]),
            "w1b": np.ascontiguousarray(w1q[:, ms:me]),
            "b1": np.ascontiguousarray(np.asarray(b1, f)[ms:me]),
            "w2b": np.ascontiguousarray(w2q[ms:me, :]),
            "b2": np.ascontiguousarray(np.asarray(b2, f)[cs:ce]),
            "wsc": wsc,
        })

    dbg = bool(os.environ.get("KERNEL_DEBUG_DUMPS"))
    nc = _get_nc(dbg)
    res = run_bass_kernel_spmd(nc, in_maps, core_ids=list(range(NCORE)))
    last_results = res
    outT = np.concatenate([res.results[c]["out"] for c in range(NCORE)], axis=0)
    return np.ascontiguousarray(outT.T).reshape(1, S, DM).astype(np.float32)

